# revision 1
# baseline (speedup 1.0000x reference)
"""Grouped-Query Attention block (RMSNorm + RoPE + causal GQA + o_proj) on 8 trn2 NeuronCores.

Sharding: data-parallel over batch (2) x tensor-parallel over kv-head groups (4).
Core c = b*4 + g handles batch b, kv heads {2g, 2g+1}, q heads {8g..8g+7}.
Each core computes a partial o_proj output (T, D) over its 768 head-dims;
host sums the 4 group partials per batch.

Layout tricks (host-side prep, all free):
  * All weights pre-transposed on host so every matmul operand DMAs naturally.
  * Per-head even/odd RoPE interleave is folded into the wq/wk weights as a
    row permutation, padded to 128 partitions: evens at rows 0:48, odds at
    64:112 (engine partition starts must be multiples of 32), pad rows are
    zero so RMSNorm sums and QK^T contractions are exact.
  * Scores are computed transposed (k on partitions, q free) so softmax's
    k-sum folds into the P@V matmul via an appended ones-column on V, and no
    big transposes are ever needed.
  * Causal mask applied structurally: above-diagonal k-tiles are skipped,
    diagonal tiles masked with affine_select (fill 0 post-exp).
  * fp32 data, fp32r matmuls (full PE rate at free-dim >= 256).
"""

import os
import sys

import numpy as np

sys.path.insert(0, "/opt/trn_rl_repo")

B, T, D = 2, 1024, 3072
NH, NKV, HD = 32, 8, 96
G = 4                 # tensor-parallel groups
QH = NH // G          # q heads per core (8)
KVH = NKV // G        # kv heads per core (2)
QPK = QH // KVH // 2  # q heads per kv head (4) -- NH/NKV
NCORES = 8
EPS = 1e-6
SCALE = 1.0 / float(np.sqrt(HD))
KT = D // 128         # 24 contraction tiles over d_model
TH = 2                # token halves in phase 1
THS = T // TH         # 512
QC = 2                # q chunks in phase 2
QCS = T // QC         # 512
KTOK = T // 128       # 8 k tiles over tokens
NJ = D // 512         # 6 output column chunks

_BUILD_CACHE = {}


def _build_nc():
    from contextlib import ExitStack
    from concourse import bacc, tile, mybir

    f32 = mybir.dt.float32
    f32r = mybir.dt.float32r
    AF = mybir.ActivationFunctionType

    nc = bacc.Bacc("TRN2", target_bir_lowering=False, debug=False,
                   num_devices=NCORES)

    xt_d = nc.dram_tensor("xt", (128, KT, T), f32r, kind="ExternalInput").ap()
    wqt_d = nc.dram_tensor("wqt", (QH, 128, KT, 128), f32r, kind="ExternalInput").ap()
    wkt_d = nc.dram_tensor("wkt", (KVH, 128, KT, 128), f32r, kind="ExternalInput").ap()
    wvt_d = nc.dram_tensor("wvt", (KVH, 128, KT, HD), f32r, kind="ExternalInput").ap()
    wot_d = nc.dram_tensor("wot", (QH, HD, D), f32r, kind="ExternalInput").ap()
    taba_d = nc.dram_tensor("taba", (128, T), f32, kind="ExternalInput").ap()
    tabb_d = nc.dram_tensor("tabb", (128, T), f32, kind="ExternalInput").ap()
    qnw_d = nc.dram_tensor("qnw", (1, 128), f32r, kind="ExternalInput").ap()
    o128_d = nc.dram_tensor("o128", (128, 1), f32r, kind="ExternalInput").ap()
    o196_d = nc.dram_tensor("o196", (1, HD), f32r, kind="ExternalInput").ap()
    ocol_d = nc.dram_tensor("ocol", (128, KTOK), f32r, kind="ExternalInput").ap()
    knw_d = nc.dram_tensor("knw", (1, 128), f32r, kind="ExternalInput").ap()
    out_d = nc.dram_tensor("out", (T, D), f32, kind="ExternalOutput").ap()

    with tile.TileContext(nc) as tc:
        with nc.allow_low_precision(reason="fp32r tiles view as fp32"), \
             ExitStack() as ctx:
            const = ctx.enter_context(tc.tile_pool(name="const", bufs=1))
            p_qkv = ctx.enter_context(tc.tile_pool(name="p_qkv", bufs=1))

            ident = const.tile([128, 128], f32, tag="ident")
            from concourse.masks import make_identity
            make_identity(nc, ident[:])
            eps_t = const.tile([1, 1], f32, tag="eps")
            nc.vector.memset(eps_t[:], EPS)
            ones128 = const.tile([128, 1], f32r, tag="ones128")
            nc.sync.dma_start(ones128[:], o128_d[:])
            ones196 = const.tile([1, HD], f32r, tag="ones196")
            nc.sync.dma_start(ones196[:], o196_d[:])

            qt = [p_qkv.tile([128, T], f32r, tag=f"qt{h}", name=f"qt{h}")
                  for h in range(QH)]
            ktl = [p_qkv.tile([128, T], f32r, tag=f"kt{g2}", name=f"kt{g2}")
                   for g2 in range(KVH)]
            vext = [p_qkv.tile([128, KTOK, HD + 1], f32r, tag=f"vx{g2}",
                               name=f"vx{g2}") for g2 in range(KVH)]
            for g2 in range(KVH):
                nc.sync.dma_start(vext[g2][:, :, HD:HD + 1], ocol_d[:])

            # ---------------- Phase 1: projections + RMSNorm + RoPE ---------
            with ExitStack() as s1:
                xt_pool = s1.enter_context(tc.tile_pool(name="xt", bufs=1))
                w_pool = s1.enter_context(tc.tile_pool(name="wst", bufs=2))
                tmp_pool = s1.enter_context(tc.tile_pool(name="tmp1", bufs=2))
                ps_pool = s1.enter_context(
                    tc.tile_pool(name="ps1", bufs=2, space="PSUM"))
                ssq_pool = s1.enter_context(
                    tc.tile_pool(name="ssq", bufs=1, space="PSUM"))
                bc_pool = s1.enter_context(
                    tc.tile_pool(name="bc1", bufs=1, space="PSUM"))
                vtr_pool = s1.enter_context(
                    tc.tile_pool(name="vtr", bufs=1, space="PSUM"))

                tab_pool = s1.enter_context(tc.tile_pool(name="tabs", bufs=1))
                taba_t = tab_pool.tile([128, T], f32, tag="taba")
                nc.sync.dma_start(taba_t[:], taba_d[:])
                tabb_t = tab_pool.tile([128, T], f32, tag="tabb")
                nc.sync.dma_start(tabb_t[:], tabb_d[:])
                qnw_t = tab_pool.tile([1, 128], f32r, tag="qnw")
                nc.sync.dma_start(qnw_t[:], qnw_d[:])
                knw_t = tab_pool.tile([1, 128], f32r, tag="knw")
                nc.sync.dma_start(knw_t[:], knw_d[:])

                # outputs ordered so attention for early heads can overlap
                outs = [("k", 0), ("v", 0), ("q", 0), ("q", 1), ("q", 2),
                        ("q", 3), ("k", 1), ("v", 1), ("q", 4), ("q", 5),
                        ("q", 6), ("q", 7)]

                for th in range(TH):
                    tsl = slice(th * THS, (th + 1) * THS)
                    xt_t = xt_pool.tile([128, KT, THS], f32r, tag="xth")
                    for kt in range(KT):
                        nc.sync.dma_start(xt_t[:, kt, :], xt_d[:, kt, tsl])
                    for kind, idx in outs:
                        if kind == "q":
                            w_t = w_pool.tile([128, KT, 128], f32r, tag="w")
                            nc.sync.dma_start(w_t[:], wqt_d[idx])
                            mdim = 128
                        elif kind == "k":
                            w_t = w_pool.tile([128, KT, 128], f32r, tag="w")
                            nc.sync.dma_start(w_t[:], wkt_d[idx])
                            mdim = 128
                        else:
                            w_t = w_pool.tile([128, KT, HD], f32r, tag="w")
                            nc.sync.dma_start(w_t[:], wvt_d[idx])
                            mdim = HD
                        ps = ps_pool.tile([128, THS], f32, tag="ps")
                        pso = ps[0:mdim, :]
                        for kt in range(KT):
                            nc.tensor.matmul(
                                pso,
                                w_t[:, kt, :],
                                xt_t[:, kt, :],
                                start=(kt == 0), stop=(kt == KT - 1))
                        if kind == "v":
                            vt = tmp_pool.tile([HD, THS], f32, tag="vt")
                            nc.scalar.copy(vt[:], pso)
                            for c in range(THS // 128):
                                tp = vtr_pool.tile([128, HD], f32, tag="tp")
                                nc.tensor.transpose(
                                    tp[:], vt[:, c * 128:(c + 1) * 128],
                                    ident[0:HD, 0:HD])
                                kidx = th * (THS // 128) + c
                                nc.scalar.copy(vext[idx][:, kidx, 0:HD], tp[:])
                            continue
                        # q/k: RMSNorm over head dim (partitions) + RoPE
                        dst = qt[idx] if kind == "q" else ktl[idx]
                        nw = qnw_t if kind == "q" else knw_t
                        sq = tmp_pool.tile([128, THS], f32r, tag="sq")
                        nc.scalar.square(sq[:], ps[:])
                        ssq = ssq_pool.tile([1, THS], f32, tag="ssq")
                        nc.tensor.matmul(ssq[:], ones128[:],
                                         sq[:],
                                         start=True, stop=True)
                        rms = tmp_pool.tile([1, THS], f32, tag="rms")
                        nc.scalar.activation(rms[:], ssq[:], AF.Sqrt,
                                             bias=eps_t[:], scale=1.0 / HD)
                        rinv = tmp_pool.tile([1, THS], f32r, tag="rinv")
                        nc.vector.reciprocal(rinv[:], rms[:])
                        bc = bc_pool.tile([128, THS], f32, tag="bc")
                        nc.tensor.matmul(bc[:], nw[:],
                                         rinv[:],
                                         start=True, stop=True)
                        bcs = tmp_pool.tile([128, THS], f32, tag="bcs")
                        nc.scalar.copy(bcs[:], bc[:])
                        sl = dst[:, tsl]
                        nc.vector.tensor_mul(sl, ps[:], bcs[:])
                        # swap halves via SBUF->SBUF DMA so every elementwise
                        # op below is partition-aligned (HW verifier requires
                        # same start partition on all tensor_tensor operands)
                        qsh = tmp_pool.tile([128, THS], f32r, tag="qsh")
                        nc.sync.dma_start(qsh[0:48, :], dst[64:112, tsl])
                        nc.sync.dma_start(qsh[64:112, :], dst[0:48, tsl])
                        # qsh[0:48] = odds(b), qsh[64:112] = evens(a)
                        tm1 = tmp_pool.tile([128, THS], f32, tag="tm1")
                        tm2 = tmp_pool.tile([128, THS], f32, tag="tm2")
                        nc.vector.tensor_mul(tm1[0:48, :], sl[0:48, :],
                                             taba_t[0:48, tsl])   # a*ce
                        nc.vector.tensor_mul(tm2[0:48, :], qsh[0:48, :],
                                             tabb_t[0:48, tsl])   # b*se
                        nc.vector.tensor_mul(tm1[64:112, :], qsh[64:112, :],
                                             taba_t[64:112, tsl])  # a*so
                        nc.vector.tensor_mul(tm2[64:112, :], sl[64:112, :],
                                             tabb_t[64:112, tsl])  # b*co
                        nc.vector.tensor_sub(dst[0:48, tsl], tm1[0:48, :],
                                             tm2[0:48, :])
                        nc.vector.tensor_add(dst[64:112, tsl],
                                             tm1[64:112, :], tm2[64:112, :])

            # ---------------- Phase 2: attention ---------------------------
            p_at = ctx.enter_context(tc.tile_pool(name="p_at", bufs=1))
            at = [p_at.tile([HD, T], f32r, tag=f"at{h}", name=f"at{h}")
                  for h in range(QH)]
            wot_t = [p_at.tile([HD, D], f32r, tag=f"wo{h}", name=f"wo{h}")
                     for h in range(QH)]
            for h in range(QH):
                nc.sync.dma_start(wot_t[h][:], wot_d[h])

            with ExitStack() as s2:
                pt_pool = s2.enter_context(tc.tile_pool(name="pt", bufs=4))
                tmp2_pool = s2.enter_context(tc.tile_pool(name="tmp2", bufs=2))
                sc_pool = s2.enter_context(
                    tc.tile_pool(name="sc", bufs=2, space="PSUM"))
                po_pool = s2.enter_context(
                    tc.tile_pool(name="po", bufs=2, space="PSUM"))
                bc2_pool = s2.enter_context(
                    tc.tile_pool(name="bc2", bufs=1, space="PSUM"))

                for h in range(QH):
                    g2 = h // (QH // KVH)
                    for qc in range(QC):
                        qsl = slice(qc * QCS, (qc + 1) * QCS)
                        nkt = (qc + 1) * QCS // 128
                        po = po_pool.tile([HD + 1, QCS], f32, tag="po")
                        for kt2 in range(nkt):
                            sc = sc_pool.tile([128, QCS], f32, tag="sc")
                            nc.tensor.matmul(
                                sc[:],
                                ktl[g2][:, kt2 * 128:(kt2 + 1) * 128],
                                qt[h][:, qsl],
                                start=True, stop=True)
                            pt = pt_pool.tile([128, QCS], f32r, tag="pt")
                            nc.scalar.activation(pt[:], sc[:], AF.Exp,
                                                 scale=SCALE)
                            if kt2 >= qc * (QCS // 128):
                                nc.gpsimd.affine_select(
                                    pt[:], pt[:], pattern=[[1, QCS]],
                                    compare_op=mybir.AluOpType.is_ge,
                                    fill=0.0,
                                    base=qc * QCS - kt2 * 128,
                                    channel_multiplier=-1)
                            nc.tensor.matmul(
                                po[:], vext[g2][:, kt2, :],
                                pt[:],
                                start=(kt2 == 0), stop=(kt2 == nkt - 1))
                        rinv2 = tmp2_pool.tile([1, QCS], f32r, tag="rinv2")
                        nc.vector.reciprocal(rinv2[:], po[HD:HD + 1, :])
                        bc2 = bc2_pool.tile([HD, QCS], f32, tag="bc2")
                        nc.tensor.matmul(bc2[:], ones196[:],
                                         rinv2[:],
                                         start=True, stop=True)
                        bc2s = tmp2_pool.tile([HD, QCS], f32, tag="bc2s")
                        nc.scalar.copy(bc2s[:], bc2[:])
                        nc.vector.tensor_mul(at[h][:, qsl], po[0:HD, :],
                                             bc2s[:])

            # ---------------- Phase 3: o_proj partial -----------------------
            with ExitStack() as s3:
                ob_pool = s3.enter_context(tc.tile_pool(name="ob", bufs=4))
                ps3_pool = s3.enter_context(
                    tc.tile_pool(name="ps3", bufs=4, space="PSUM"))
                for i in range(T // 128):
                    isl = slice(i * 128, (i + 1) * 128)
                    for j in range(NJ):
                        jsl = slice(j * 512, (j + 1) * 512)
                        ps3 = ps3_pool.tile([128, 512], f32, tag="ps3")
                        for h in range(QH):
                            nc.tensor.matmul(
                                ps3[:], at[h][:, isl],
                                wot_t[h][:, jsl],
                                start=(h == 0), stop=(h == QH - 1))
                        ob = ob_pool.tile([128, 512], f32, tag="ob")
                        nc.scalar.copy(ob[:], ps3[:])
                        nc.sync.dma_start(out_d[isl, jsl], ob[:])

    nc.compile()
    return nc


def get_nc():
    if "nc" not in _BUILD_CACHE:
        _BUILD_CACHE["nc"] = _build_nc()
    return _BUILD_CACHE["nc"]


def _permpad_rows(w96):
    """(96, N) head rows -> (128, N): evens at 0:48, odds at 64:112, pad 0."""
    out = np.zeros((128, w96.shape[1]), np.float32)
    out[0:48] = w96[0::2]
    out[64:112] = w96[1::2]
    return out


def _lhsT_tiles(wT, m):
    """(D, m) -> (128, KT, m) lhsT tile layout (contraction on partitions)."""
    return np.ascontiguousarray(
        wT.reshape(KT, 128, m).transpose(1, 0, 2)).astype(np.float32)


def prepare_in_maps(x, wq, wk, wv, wo, q_norm_w, k_norm_w, cos, sin):
    x = np.asarray(x, np.float32)
    wq = np.asarray(wq, np.float32)
    wk = np.asarray(wk, np.float32)
    wv = np.asarray(wv, np.float32)
    wo = np.asarray(wo, np.float32)
    cos = np.asarray(cos, np.float32)
    sin = np.asarray(sin, np.float32)
    q_norm_w = np.asarray(q_norm_w, np.float32)
    k_norm_w = np.asarray(k_norm_w, np.float32)

    taba = np.zeros((128, T), np.float32)
    tabb = np.zeros((128, T), np.float32)
    taba[0:48] = cos[:, 0::2].T
    taba[64:112] = sin[:, 1::2].T
    tabb[0:48] = sin[:, 0::2].T
    tabb[64:112] = cos[:, 1::2].T
    qnw = np.zeros((1, 128), np.float32)
    qnw[0, 0:48] = q_norm_w[0::2]
    qnw[0, 64:112] = q_norm_w[1::2]
    knw = np.zeros((1, 128), np.float32)
    knw[0, 0:48] = k_norm_w[0::2]
    knw[0, 64:112] = k_norm_w[1::2]

    xts = []
    for b in range(B):
        xT = np.ascontiguousarray(x[b].T)  # (D, T)
        xts.append(np.ascontiguousarray(
            xT.reshape(KT, 128, T).transpose(1, 0, 2)))

    in_maps = []
    for c in range(NCORES):
        b, g = divmod(c, G)
        wqt = np.stack([
            _lhsT_tiles(_permpad_rows(
                wq[(g * QH + i) * HD:(g * QH + i + 1) * HD]).T, 128)
            for i in range(QH)])
        wkt = np.stack([
            _lhsT_tiles(_permpad_rows(
                wk[(g * KVH + i) * HD:(g * KVH + i + 1) * HD]).T, 128)
            for i in range(KVH)])
        wvt = np.stack([
            _lhsT_tiles(np.ascontiguousarray(
                wv[(g * KVH + i) * HD:(g * KVH + i + 1) * HD].T), HD)
            for i in range(KVH)])
        wo_sh = wo[:, g * QH * HD:(g + 1) * QH * HD]  # (D, 768)
        wot = np.stack([
            np.ascontiguousarray(wo_sh[:, i * HD:(i + 1) * HD].T)
            for i in range(QH)])
        in_maps.append({
            "xt": xts[b], "wqt": wqt, "wkt": wkt, "wvt": wvt, "wot": wot,
            "taba": taba, "tabb": tabb, "qnw": qnw, "knw": knw,
            "o128": np.ones((128, 1), np.float32),
            "o196": np.ones((1, HD), np.float32),
            "ocol": np.ones((128, KTOK), np.float32),
        })
    return in_maps


def kernel(**inputs):
    from concourse import bass_utils

    nc = get_nc()
    in_maps = prepare_in_maps(
        inputs["x"], inputs["wq"], inputs["wk"], inputs["wv"], inputs["wo"],
        inputs["q_norm_w"], inputs["k_norm_w"], inputs["cos"], inputs["sin"])
    trace = bool(int(os.environ.get("BASS_KERNEL_TRACE", "0")))
    res = bass_utils.run_bass_kernel_spmd(
        nc, in_maps, core_ids=list(range(NCORES)), trace=trace)
    _BUILD_CACHE["last_result"] = res
    partials = [np.asarray(r["out"]) for r in res.results]
    out = np.empty((B, T, D), np.float32)
    for b in range(B):
        out[b] = np.sum(np.stack(partials[b * G:(b + 1) * G]), axis=0,
                        dtype=np.float64).astype(np.float32)
    return out



# revision 24
# speedup vs baseline: 1.5643x; 1.5643x over previous
"""Grouped-Query Attention block (RMSNorm + RoPE + causal GQA + o_proj) on 8 trn2 NeuronCores.

Sharding: data-parallel over batch (2) x tensor-parallel over kv-head groups (4).
Core c = b*4 + g handles batch b, kv heads {2g, 2g+1}, q heads {8g..8g+7}.
Each core computes a partial o_proj output (T, D) over its 768 head-dims;
host sums the 4 group partials per batch.

v2 design (vs v1):
  * Phase 1 projections in token-on-partition layout: x tile is the
    stationary operand, all 12 head outputs (8q+2k+2v = 1152 dims) are the
    moving free dim in 3 dense 384-wide chunks -> 25% fewer PE cycles than
    per-head padded outputs, and RMSNorm/RoPE become cheap free-dim ops
    (per-token scalars live on partitions: tensor_scalar / [128,10]
    reciprocals instead of [1,512] single-lane reciprocals + PE broadcasts).
  * bf16 everywhere on the matmul path (same PE rate as fp32r at these
    free sizes, half the DMA/SBUF traffic).
  * q/k transposed back to [head_dim, T] via PE transpose-mode (96x128).
  * v needs no transpose at all in this layout (it was 16 PE transposes in v1).
  * Attention: scores transposed (k on partitions) with softmax k-sum folded
    into PV via per-head ones-columns appended to V at column 96+h, so each
    head's denominator lands on its own partition -> batched [8,512]
    reciprocal instead of 16 serial [1,512] ones (120us of DVE in v1).
  * Causal mask applied structurally (skip above-diagonal k-tiles, gpsimd
    affine_select on the rest) - same as v1.
  * Phase 3 o_proj over a densely packed [768 = 6x128, T] activation
    (head boundaries straddle tiles; packing via SBUF-SBUF DMA) -> 6
    contraction tiles instead of 8 per output tile: 25% fewer PE cycles.
  * Emission is software-pipelined so the PE queue never head-blocks on
    the ACT/DVE/gpsimd post-processing chains: transposes for token-tile
    tt are emitted after the projections of tt+1; attention q-chunk 0 is
    interleaved into the phase-1 tail; o_proj i-blocks are interleaved
    into attention q-chunk 1.
"""

import os
import sys

import numpy as np

sys.path.insert(0, "/opt/trn_rl_repo")

B, T, D = 2, 1024, 3072
NH, NKV, HD = 32, 8, 96
G = 4                 # tensor-parallel groups
QH = NH // G          # q heads per core (8)
KVH = NKV // G        # kv heads per core (2)
NCORES = 8
EPS = 1e-6
SCALE = 1.0 / float(np.sqrt(HD))
KT = D // 128          # 24 contraction tiles over d_model
NTT = T // 128         # 8 token tiles
QC = 2                 # q chunks in phase 2
QCS = T // QC          # 512
NJ = D // 512          # 6 o_proj output column chunks
RH = QH + KVH          # 10 rope heads (8 q + 2 k)
PRJ = QH * HD + KVH * HD * 2   # 1152 projected dims per core
NCH = 3                # projection chunks
CHW = PRJ // NCH       # 384
VCOL = HD + QH         # 104: v columns + per-head ones columns

_BUILD_CACHE = {}


def _build_nc():
    from contextlib import ExitStack
    from concourse import bacc, tile, mybir

    f32 = mybir.dt.float32
    f32r = mybir.dt.float32r
    bf16 = mybir.dt.bfloat16
    AF = mybir.ActivationFunctionType
    ALU = mybir.AluOpType

    nc = bacc.Bacc("TRN2", target_bir_lowering=False, debug=False,
                   num_devices=NCORES)

    xt_d = nc.dram_tensor("xt", (128, NTT, KT, 128), bf16, kind="ExternalInput").ap()
    w_d = nc.dram_tensor("w", (128, KT, PRJ), bf16, kind="ExternalInput").ap()
    wot_d = nc.dram_tensor("wot", (128, 6, D), bf16, kind="ExternalInput").ap()
    tabaq_d = nc.dram_tensor("tabaq", (128, NTT, HD), bf16, kind="ExternalInput").ap()
    tabbqe_d = nc.dram_tensor("tabbqe", (128, NTT, HD // 2), bf16, kind="ExternalInput").ap()
    tabbqo_d = nc.dram_tensor("tabbqo", (128, NTT, HD // 2), bf16, kind="ExternalInput").ap()
    tabak_d = nc.dram_tensor("tabak", (128, NTT, HD), bf16, kind="ExternalInput").ap()
    tabbke_d = nc.dram_tensor("tabbke", (128, NTT, HD // 2), bf16, kind="ExternalInput").ap()
    tabbko_d = nc.dram_tensor("tabbko", (128, NTT, HD // 2), bf16, kind="ExternalInput").ap()
    out_d = nc.dram_tensor("out", (T, D), bf16, kind="ExternalOutput").ap()

    with tile.TileContext(nc) as tc:
        with nc.allow_low_precision(reason="bf16 matmul path, fp32 accum"), \
             ExitStack() as ctx:
            const = ctx.enter_context(tc.tile_pool(name="const", bufs=1))

            ident = const.tile([128, 128], bf16, tag="ident")
            from concourse.masks import make_identity
            make_identity(nc, ident[:])
            eps_t = const.tile([128, 1], f32, tag="eps")
            nc.vector.memset(eps_t[:], EPS)
            ones_t = const.tile([128, HD], bf16, tag="ones_t")
            nc.vector.memset(ones_t[:], 1.0)

            # rope/norm tables (token-tile layout)
            tabs = ctx.enter_context(tc.tile_pool(name="tabs", bufs=1))
            tabaq = tabs.tile([128, NTT, HD], bf16, tag="tabaq")
            nc.sync.dma_start(tabaq[:], tabaq_d[:])
            tabbqe = tabs.tile([128, NTT, HD // 2], bf16, tag="tabbqe")
            nc.sync.dma_start(tabbqe[:], tabbqe_d[:])
            tabbqo = tabs.tile([128, NTT, HD // 2], bf16, tag="tabbqo")
            nc.sync.dma_start(tabbqo[:], tabbqo_d[:])
            tabak = tabs.tile([128, NTT, HD], bf16, tag="tabak")
            nc.sync.dma_start(tabak[:], tabak_d[:])
            tabbke = tabs.tile([128, NTT, HD // 2], bf16, tag="tabbke")
            nc.sync.dma_start(tabbke[:], tabbke_d[:])
            tabbko = tabs.tile([128, NTT, HD // 2], bf16, tag="tabbko")
            nc.sync.dma_start(tabbko[:], tabbko_d[:])

            # weights resident in SBUF
            wpool = ctx.enter_context(tc.tile_pool(name="wpool", bufs=1))
            w_sb = wpool.tile([128, KT, PRJ], bf16, tag="w_sb")
            for kt in range(KT):
                nc.sync.dma_start(w_sb[:, kt, :], w_d[:, kt, :])
            wot_sb = wpool.tile([128, 6, D], bf16, tag="wot_sb")
            for t6 in range(6):
                nc.sync.dma_start(wot_sb[:, t6, :], wot_d[:, t6, :])

            # persistent activations
            p_act = ctx.enter_context(tc.tile_pool(name="p_act", bufs=1))
            qt = [p_act.tile([HD, T], bf16, tag=f"qt{h}", name=f"qt{h}")
                  for h in range(QH)]
            ktl = [p_act.tile([HD, T], bf16, tag=f"kt{g2}", name=f"kt{g2}")
                   for g2 in range(KVH)]
            vext = [p_act.tile([128, NTT, VCOL], bf16, tag=f"vx{g2}",
                               name=f"vx{g2}") for g2 in range(KVH)]
            for g2 in range(KVH):
                nc.vector.memset(vext[g2][:, :, HD:VCOL], 1.0)
            at_dense = [p_act.tile([128, 6, QCS], bf16, tag=f"at{qc}",
                                   name=f"at{qc}") for qc in range(QC)]
            sums_sb = p_act.tile([128, T], f32r, tag="sums_sb")
            rinv_sb = p_act.tile([128, T], bf16, tag="rinv_sb")
            # per-qc staging of rinv rows at partition 0 (reused across qc)
            rl8 = p_act.tile([1, QH, QCS], bf16, tag="rl8")
            amun = p_act.tile([128, QC * QH, QCS], bf16, tag="amun")

            # ---------------- pools ----------------
            # pool release must be LIFO: s2 (lives longest) is created
            # before s1 (phase-1 scope, closed first).
            s2 = ExitStack()
            sc_pool = s2.enter_context(
                tc.tile_pool(name="sc", bufs=2, space="PSUM"))
            po_pool = s2.enter_context(
                tc.tile_pool(name="po", bufs=1, space="PSUM"))
            bc2_pool = s2.enter_context(
                tc.tile_pool(name="bc2", bufs=1, space="PSUM"))
            pt_pool = s2.enter_context(tc.tile_pool(name="pt", bufs=3))
            am_pool = s2.enter_context(tc.tile_pool(name="am", bufs=2))

            s1 = ExitStack()
            xt_pool = s1.enter_context(tc.tile_pool(name="xt", bufs=2))
            ps1_pool = s1.enter_context(
                tc.tile_pool(name="ps1", bufs=1, space="PSUM"))
            pst_pool = s1.enter_context(
                tc.tile_pool(name="pst", bufs=1, space="PSUM"))
            w1_pool = s1.enter_context(tc.tile_pool(name="w1", bufs=2))

            # ---------------- phase 1 pieces ----------------
            psx_tiles = {}

            def p1_mms(tt):
                xt_t = xt_pool.tile([128, KT, 128], bf16, tag="xt_t",
                                    name=f"xt{tt}")
                nc.sync.dma_start(xt_t[:], xt_d[:, tt])
                pss = []
                for c in range(NCH):
                    ps = ps1_pool.tile([128, CHW], f32, tag=f"ps{c}",
                                       name=f"ps{c}_{tt}")
                    pss.append(ps)
                for kt in range(KT):
                    for c in range(NCH):
                        nc.tensor.matmul(
                            pss[c][:],
                            xt_t[:, kt, :],
                            w_sb[:, kt, c * CHW:(c + 1) * CHW],
                            start=(kt == 0), stop=(kt == KT - 1))
                psx_tiles[tt] = pss

            roped_tiles = {}

            def p1_post(tt):
                pss = psx_tiles[tt]
                # copy psum -> sbuf bf16 (frees psum for next tt quickly)
                prj = w1_pool.tile([128, NCH, CHW], bf16, tag="prj",
                                   name=f"prj{tt}")
                for c in range(NCH):
                    nc.scalar.copy(prj[:, c, :], pss[c][:])
                # v heads: straight copy into vext (token-layout, no transpose)
                for g2 in range(KVH):
                    nc.gpsimd.tensor_copy(
                        vext[g2][:, tt, 0:HD],
                        prj[:, 2, HD * (KVH + g2):HD * (KVH + g2 + 1)])
                # rms: per-head sum of squares along free dim
                ssq = w1_pool.tile([128, RH], f32, tag="ssq", name=f"ssq{tt}")
                sqs = w1_pool.tile([128, HD], bf16, tag="sqs", name=f"sqs{tt}")
                for rh in range(RH):
                    c, off = divmod(rh * HD, CHW)
                    sl = prj[:, c, off:off + HD]
                    nc.vector.scalar_tensor_tensor(
                        sqs[:], sl, 1.0, sl,
                        op0=ALU.mult, op1=ALU.mult,
                        accum_out=ssq[:, rh:rh + 1])
                rms = w1_pool.tile([128, RH], f32, tag="rms", name=f"rms{tt}")
                nc.scalar.activation(rms[:], ssq[:], AF.Sqrt,
                                     bias=eps_t[:], scale=1.0 / HD)
                rinv = w1_pool.tile([128, RH], bf16, tag="rinv",
                                    name=f"rinv{tt}")
                nc.vector.reciprocal(rinv[:], rms[:])
                # rope (batched across heads per chunk)
                roped = w1_pool.tile([128, RH * HD], bf16, tag="roped",
                                     name=f"roped{tt}")
                tm0 = w1_pool.tile([128, CHW], bf16, tag="tm0", name=f"tm0{tt}")
                tm1 = w1_pool.tile([128, CHW], bf16, tag="tm1", name=f"tm1{tt}")
                tm2 = w1_pool.tile([128, CHW], bf16, tag="tm2", name=f"tm2{tt}")
                for c in range(NCH):
                    nh = 4 if c < 2 else 2
                    w96 = nh * HD
                    taba, tbe, tbo = ((tabaq, tabbqe, tabbqo) if c < 2
                                      else (tabak, tabbke, tabbko))
                    src = prj[:, c, 0:w96].rearrange("p (h d) -> p h d", h=nh)
                    rb = (rinv[:, c * 4:c * 4 + nh]
                          .unsqueeze(2).broadcast_to([128, nh, HD]))
                    t0 = tm0[:, 0:w96].rearrange("p (h d) -> p h d", h=nh)
                    nc.vector.tensor_mul(t0, src, rb)
                    ta = (taba[:, tt, :].unsqueeze(1)
                          .broadcast_to([128, nh, HD]))
                    t1 = tm1[:, 0:w96].rearrange("p (h d) -> p h d", h=nh)
                    nc.vector.tensor_mul(t1, t0, ta)
                    t0p = tm0[:, 0:w96].rearrange("p (h i e) -> p h i e",
                                                  h=nh, e=2)
                    t2p = tm2[:, 0:w96].rearrange("p (h i e) -> p h i e",
                                                  h=nh, e=2)
                    tbe_b = (tbe[:, tt, :].unsqueeze(1)
                             .broadcast_to([128, nh, HD // 2]))
                    tbo_b = (tbo[:, tt, :].unsqueeze(1)
                             .broadcast_to([128, nh, HD // 2]))
                    nc.vector.tensor_mul(t2p[:, :, :, 0], t0p[:, :, :, 1],
                                         tbe_b)
                    nc.vector.tensor_mul(t2p[:, :, :, 1], t0p[:, :, :, 0],
                                         tbo_b)
                    dst = roped[:, c * CHW:c * CHW + w96]
                    nc.vector.tensor_add(dst, tm1[:, 0:w96], tm2[:, 0:w96])
                roped_tiles[tt] = roped

            def p1_transposes(tt):
                roped = roped_tiles[tt]
                for rh in range(RH):
                    pst = pst_pool.tile([128, 128], bf16, tag="pst",
                                        name=f"pst{tt}_{rh}")
                    nc.tensor.transpose(
                        pst[0:HD, :],
                        roped[:, rh * HD:(rh + 1) * HD],
                        ident[:])
                    dst = qt[rh] if rh < QH else ktl[rh - QH]
                    nc.scalar.copy(dst[:, tt * 128:(tt + 1) * 128],
                                   pst[0:HD, :])

            # ---------------- phase 2 pieces ----------------
            def p2_head(h, qc):
                g2 = h // (QH // KVH)
                qsl = slice(qc * QCS, (qc + 1) * QCS)
                nkt = (qc + 1) * QCS // 128
                po = po_pool.tile([128, QCS], f32, tag="po",
                                  name=f"po{qc}_{h}")
                poa = po[0:HD + h + 1, :]
                pts = {}

                def sc_mm(kt2):
                    sc = sc_pool.tile([128, QCS], f32, tag="sc",
                                      name=f"sc{qc}_{h}_{kt2}")
                    nc.tensor.matmul(
                        sc[:],
                        ktl[g2][:, kt2 * 128:(kt2 + 1) * 128],
                        qt[h][:, qsl],
                        start=True, stop=True)
                    return sc

                def pt_make(kt2, sc):
                    pt = pt_pool.tile([128, QCS], bf16, tag="pt",
                                      name=f"pt{qc}_{h}_{kt2}")
                    nc.scalar.activation(pt[:], sc[:], AF.Exp, scale=SCALE)
                    if kt2 >= qc * (QCS // 128):
                        nc.gpsimd.affine_select(
                            pt[:], pt[:], pattern=[[1, QCS]],
                            compare_op=ALU.is_ge,
                            fill=0.0,
                            base=qc * QCS - kt2 * 128,
                            channel_multiplier=-1)
                    pts[kt2] = pt

                def pv_mm(kt2):
                    nc.tensor.matmul(
                        poa, vext[g2][:, kt2, 0:HD + h + 1],
                        pts[kt2][:],
                        start=(kt2 == 0), stop=(kt2 == nkt - 1))
                    pts[kt2] = None

                # software-pipelined: keep 2 sc tiles in flight
                sc0 = sc_mm(0)
                pt_make(0, sc0)
                for kt2 in range(1, nkt):
                    sc_n = sc_mm(kt2)
                    pv_mm(kt2 - 1)
                    pt_make(kt2, sc_n)
                pv_mm(nkt - 1)
                # denominator: po rows 96..96+h all hold this head's sum.
                # Engine partition starts must be 32-aligned, so copy the
                # whole [96:97+h] block; heads are processed in DESCENDING
                # order so each later (smaller) copy leaves row 96+h' of
                # earlier heads h' > h intact.
                nc.scalar.copy(sums_sb[HD:HD + h + 1, qsl],
                               po[HD:HD + h + 1, :])
                nc.vector.tensor_copy(amun[0:HD, qc * QH + h, :], po[0:HD, :])

            def p2_recip(qc):
                qsl = slice(qc * QCS, (qc + 1) * QCS)
                nc.vector.reciprocal(rinv_sb[HD:HD + QH, qsl],
                                     sums_sb[HD:HD + QH, qsl])
                # matmul lhsT/rhs base partition must be in {0,32,64}:
                # shift each head's rinv row down to partition 0 via DMA
                for h in range(QH):
                    nc.sync.dma_start(rl8[0:1, h, :],
                                      rinv_sb[HD + h:HD + h + 1, qsl])

            def p2_norm(h, qc):
                qsl = slice(qc * QCS, (qc + 1) * QCS)
                bc2 = bc2_pool.tile([128, QCS], f32, tag="bc2",
                                    name=f"bc2{qc}_{h}")
                nc.tensor.matmul(
                    bc2[0:HD, :],
                    ones_t[0:1, :],
                    rl8[0:1, h, :],
                    start=True, stop=True)
                am = am_pool.tile([HD, QCS], bf16, tag="am",
                                  name=f"am{qc}_{h}")
                nc.vector.tensor_mul(am[:], amun[0:HD, qc * QH + h, :],
                                     bc2[0:HD, :])
                # scatter into dense [768 = 6x128] layout (partition shift
                # is only legal via DMA)
                r0 = h * HD
                t0, off = divmod(r0, 128)
                n1 = min(128 - off, HD)
                nc.sync.dma_start(at_dense[qc][off:off + n1, t0, :],
                                  am[0:n1, :])
                if n1 < HD:
                    nc.sync.dma_start(
                        at_dense[qc][0:HD - n1, t0 + 1, :],
                        am[n1:HD, :])

            # ---------------- phase 3 pieces ----------------
            s3 = ExitStack()
            ps3_pool = None
            ob_pool = None

            def p3_open():
                nonlocal ps3_pool, ob_pool
                ps3_pool = s3.enter_context(
                    tc.tile_pool(name="ps3", bufs=2, space="PSUM"))
                ob_pool = s3.enter_context(tc.tile_pool(name="ob", bufs=1))

            def p3_iblock(i):
                qc = i // (QCS // 128)
                isl = slice((i % (QCS // 128)) * 128,
                            (i % (QCS // 128)) * 128 + 128)
                ob = ob_pool.tile([128, NJ, 512], bf16, tag="ob",
                                  name=f"ob{i}")
                for j in range(NJ):
                    ps3 = ps3_pool.tile([128, 512], f32, tag="ps3",
                                        name=f"ps3_{i}_{j}")
                    for t6 in range(6):
                        nc.tensor.matmul(
                            ps3[:], at_dense[qc][:, t6, isl],
                            wot_sb[:, t6, j * 512:(j + 1) * 512],
                            start=(t6 == 0), stop=(t6 == 5))
                    nc.vector.tensor_copy(ob[:, j, :], ps3[:])
                nc.sync.dma_start(
                    out_d[i * 128:(i + 1) * 128, :],
                    ob[:].rearrange("p a b -> p (a b)"))

            # ---------------- emission schedule ----------------
            with s1:
                for tt in range(4):
                    p1_mms(tt)
                    if tt > 0:
                        p1_transposes(tt - 1)
                    p1_post(tt)
                for tt in range(4, NTT):
                    p1_mms(tt)
                    p1_transposes(tt - 1)
                    p1_post(tt)
                    p2_head(QH - 1 - (tt - 4), 0)
                p1_transposes(NTT - 1)
                for h in range(3, -1, -1):
                    p2_head(h, 0)
                p2_recip(0)
            # phase-1 psum freed; open phase-3 pools
            p3_open()
            with s3:
                order = []
                for pos, h in enumerate(range(QH - 1, -1, -1)):
                    order.append(("h", h))
                    if pos < 2:
                        order.append(("n", 2 * pos))
                        order.append(("n", 2 * pos + 1))
                    elif pos == 2:
                        order += [("n", 4), ("n", 5), ("n", 6), ("n", 7)]
                    elif 3 <= pos <= 6:
                        order.append(("i", pos - 3))
                for kind, v in order:
                    if kind == "h":
                        p2_head(v, 1)
                    elif kind == "n":
                        p2_norm(v, 0)
                    else:
                        p3_iblock(v)
                p2_recip(1)
                for h in range(QH):
                    p2_norm(h, 1)
                for i in range(4, NTT):
                    p3_iblock(i)
            s2.close()

    nc.compile()
    return nc


def get_nc():
    if "nc" not in _BUILD_CACHE:
        _BUILD_CACHE["nc"] = _build_nc()
    return _BUILD_CACHE["nc"]


def prepare_in_maps(x, wq, wk, wv, wo, q_norm_w, k_norm_w, cos, sin):
    import ml_dtypes
    bf = ml_dtypes.bfloat16

    x = np.asarray(x, np.float32)
    wq = np.asarray(wq, np.float32)
    wk = np.asarray(wk, np.float32)
    wv = np.asarray(wv, np.float32)
    wo = np.asarray(wo, np.float32)
    cos = np.asarray(cos, np.float32)
    sin = np.asarray(sin, np.float32)
    qw = np.asarray(q_norm_w, np.float32)
    kw = np.asarray(k_norm_w, np.float32)

    # rope tables in token-tile layout [128, NTT, ...]
    def tok_tiles(a):  # (T, F) -> (128, NTT, F)
        F = a.shape[1]
        return np.ascontiguousarray(
            a.reshape(NTT, 128, F).transpose(1, 0, 2)).astype(bf)

    tabaq = tok_tiles(cos * qw[None, :])
    tabak = tok_tiles(cos * kw[None, :])
    tabbqe = tok_tiles(-sin[:, 0::2] * qw[None, 1::2])
    tabbqo = tok_tiles(sin[:, 1::2] * qw[None, 0::2])
    tabbke = tok_tiles(-sin[:, 0::2] * kw[None, 1::2])
    tabbko = tok_tiles(sin[:, 1::2] * kw[None, 0::2])

    # x: [128, NTT, KT, 128] per batch (contraction tiles on partitions)
    xts = []
    for b in range(B):
        xT = x[b].T  # (D, T)
        # (KT,128,T) -> partitions first, then token tiles contiguous
        t1 = xT.reshape(KT, 128, NTT, 128)
        xts.append(np.ascontiguousarray(
            t1.transpose(1, 2, 0, 3)).astype(bf))

    in_maps = []
    wcache = {}
    for c in range(NCORES):
        b, g = divmod(c, G)
        if g not in wcache:
            # W columns: q heads g*8..g*8+7, then k0,k1, v0,v1 (96 each)
            cols = [wq[(g * QH + i) * HD:(g * QH + i + 1) * HD]
                    for i in range(QH)]
            cols += [wk[(g * KVH + i) * HD:(g * KVH + i + 1) * HD]
                     for i in range(KVH)]
            cols += [wv[(g * KVH + i) * HD:(g * KVH + i + 1) * HD]
                     for i in range(KVH)]
            wall = np.concatenate(cols, axis=0).T  # (D, 1152)
            w_t = np.ascontiguousarray(
                wall.reshape(KT, 128, PRJ).transpose(1, 0, 2)).astype(bf)
            # wot: rows = packed [768] head dims, cols = D
            wo_sh = wo[:, g * QH * HD:(g + 1) * QH * HD]  # (D, 768)
            wot = np.ascontiguousarray(
                wo_sh.T.reshape(6, 128, D).transpose(1, 0, 2)).astype(bf)
            wcache[g] = (w_t, wot)
        w_t, wot = wcache[g]
        in_maps.append({
            "xt": xts[b], "w": w_t, "wot": wot,
            "tabaq": tabaq, "tabbqe": tabbqe, "tabbqo": tabbqo,
            "tabak": tabak, "tabbke": tabbke, "tabbko": tabbko,
        })
    return in_maps


def kernel(**inputs):
    from concourse import bass_utils

    nc = get_nc()
    in_maps = prepare_in_maps(
        inputs["x"], inputs["wq"], inputs["wk"], inputs["wv"], inputs["wo"],
        inputs["q_norm_w"], inputs["k_norm_w"], inputs["cos"], inputs["sin"])
    trace = bool(int(os.environ.get("BASS_KERNEL_TRACE", "0")))
    res = bass_utils.run_bass_kernel_spmd(
        nc, in_maps, core_ids=list(range(NCORES)), trace=trace)
    _BUILD_CACHE["last_result"] = res
    partials = [np.asarray(r["out"], np.float32) for r in res.results]
    out = np.empty((B, T, D), np.float32)
    for b in range(B):
        out[b] = np.sum(np.stack(partials[b * G:(b + 1) * G]), axis=0,
                        dtype=np.float64).astype(np.float32)
    return out


# revision 33
# speedup vs baseline: 1.6630x; 1.0631x over previous
"""Grouped-Query Attention block (RMSNorm + RoPE + causal GQA + o_proj) on 8 trn2 NeuronCores.

Sharding: data-parallel over batch (2) x tensor-parallel over kv-head groups (4).
Core c = b*4 + g handles batch b, kv heads {2g, 2g+1}, q heads {8g..8g+7}.
Each core computes a partial o_proj output (T, D) over its 768 head-dims;
host sums the 4 group partials per batch.

v2 design (vs v1):
  * Phase 1 projections in token-on-partition layout: x tile is the
    stationary operand, all 12 head outputs (8q+2k+2v = 1152 dims) are the
    moving free dim in 3 dense 384-wide chunks -> 25% fewer PE cycles than
    per-head padded outputs, and RMSNorm/RoPE become cheap free-dim ops
    (per-token scalars live on partitions: tensor_scalar / [128,10]
    reciprocals instead of [1,512] single-lane reciprocals + PE broadcasts).
  * bf16 everywhere on the matmul path (same PE rate as fp32r at these
    free sizes, half the DMA/SBUF traffic).
  * q/k transposed back to [head_dim, T] via PE transpose-mode (96x128).
  * v needs no transpose at all in this layout (it was 16 PE transposes in v1).
  * Attention: scores transposed (k on partitions) with softmax k-sum folded
    into PV via per-head ones-columns appended to V at column 96+h, so each
    head's denominator lands on its own partition -> batched [8,512]
    reciprocal instead of 16 serial [1,512] ones (120us of DVE in v1).
  * Causal mask applied structurally (skip above-diagonal k-tiles, gpsimd
    affine_select on the rest) - same as v1.
  * Phase 3 o_proj over a densely packed [768 = 6x128, T] activation
    (head boundaries straddle tiles; packing via SBUF-SBUF DMA) -> 6
    contraction tiles instead of 8 per output tile: 25% fewer PE cycles.
  * Emission is software-pipelined so the PE queue never head-blocks on
    the ACT/DVE/gpsimd post-processing chains: transposes for token-tile
    tt are emitted after the projections of tt+1; attention q-chunk 0 is
    interleaved into the phase-1 tail; o_proj i-blocks are interleaved
    into attention q-chunk 1.
"""

import os
import sys

import numpy as np

sys.path.insert(0, "/opt/trn_rl_repo")

B, T, D = 2, 1024, 3072
NH, NKV, HD = 32, 8, 96
G = 4                 # tensor-parallel groups
QH = NH // G          # q heads per core (8)
KVH = NKV // G        # kv heads per core (2)
NCORES = 8
EPS = 1e-6
SCALE = 1.0 / float(np.sqrt(HD))
KT = D // 128          # 24 contraction tiles over d_model
NTT = T // 128         # 8 token tiles
QC = 2                 # q chunks in phase 2
QCS = T // QC          # 512
NJ = D // 512          # 6 o_proj output column chunks
RH = QH + KVH          # 10 rope heads (8 q + 2 k)
PRJ = QH * HD + KVH * HD * 2   # 1152 projected dims per core
NCH = 3                # projection chunks
CHW = PRJ // NCH       # 384
VCOL = HD + QH         # 104: v columns + per-head ones columns

_BUILD_CACHE = {}


def _build_nc():
    from contextlib import ExitStack
    from concourse import bacc, tile, mybir

    f32 = mybir.dt.float32
    f32r = mybir.dt.float32r
    bf16 = mybir.dt.bfloat16
    AF = mybir.ActivationFunctionType
    ALU = mybir.AluOpType

    nc = bacc.Bacc("TRN2", target_bir_lowering=False, debug=False,
                   num_devices=NCORES)

    xt_d = nc.dram_tensor("xt", (128, NTT, KT, 128), bf16, kind="ExternalInput").ap()
    w_d = nc.dram_tensor("w", (128, KT, PRJ), bf16, kind="ExternalInput").ap()
    wot_d = nc.dram_tensor("wot", (128, 6, D), bf16, kind="ExternalInput").ap()
    tabaq_d = nc.dram_tensor("tabaq", (128, NTT, HD), bf16, kind="ExternalInput").ap()
    tabbqe_d = nc.dram_tensor("tabbqe", (128, NTT, HD // 2), bf16, kind="ExternalInput").ap()
    tabbqo_d = nc.dram_tensor("tabbqo", (128, NTT, HD // 2), bf16, kind="ExternalInput").ap()
    tabak_d = nc.dram_tensor("tabak", (128, NTT, HD), bf16, kind="ExternalInput").ap()
    tabbke_d = nc.dram_tensor("tabbke", (128, NTT, HD // 2), bf16, kind="ExternalInput").ap()
    tabbko_d = nc.dram_tensor("tabbko", (128, NTT, HD // 2), bf16, kind="ExternalInput").ap()
    out_d = nc.dram_tensor("out", (T, D), bf16, kind="ExternalOutput").ap()

    with tile.TileContext(nc) as tc:
        with nc.allow_low_precision(reason="bf16 matmul path, fp32 accum"), \
             ExitStack() as ctx:
            const = ctx.enter_context(tc.tile_pool(name="const", bufs=1))

            ident = const.tile([128, 128], bf16, tag="ident")
            from concourse.masks import make_identity
            make_identity(nc, ident[:])
            eps_t = const.tile([128, 1], f32, tag="eps")
            nc.vector.memset(eps_t[:], EPS)
            ones_t = const.tile([128, HD], bf16, tag="ones_t")
            nc.vector.memset(ones_t[:], 1.0)

            # rope/norm tables (token-tile layout); DMA issues deferred
            tabs = ctx.enter_context(tc.tile_pool(name="tabs", bufs=1))
            tabaq = tabs.tile([128, NTT, HD], bf16, tag="tabaq")
            tabbqe = tabs.tile([128, NTT, HD // 2], bf16, tag="tabbqe")
            tabbqo = tabs.tile([128, NTT, HD // 2], bf16, tag="tabbqo")
            tabak = tabs.tile([128, NTT, HD], bf16, tag="tabak")
            tabbke = tabs.tile([128, NTT, HD // 2], bf16, tag="tabbke")
            tabbko = tabs.tile([128, NTT, HD // 2], bf16, tag="tabbko")

            def tab_dmas():
                for t, td in ((tabaq, tabaq_d), (tabbqe, tabbqe_d),
                              (tabbqo, tabbqo_d), (tabak, tabak_d),
                              (tabbke, tabbke_d), (tabbko, tabbko_d)):
                    nc.sync.dma_start(t[:], td[:])

            # weights resident in SBUF.  DMA issue order matters: the W
            # chunks are issued interleaved with the first token tiles so
            # the first projection matmuls are not stuck behind 12MB of
            # weight traffic; wot (only needed by o_proj, >200us in) is
            # issued mid-phase-1.
            wpool = ctx.enter_context(tc.tile_pool(name="wpool", bufs=1))
            w_sb = wpool.tile([128, KT, PRJ], bf16, tag="w_sb")
            wot_sb = wpool.tile([128, 6, D], bf16, tag="wot_sb")
            W_CHUNKS = [(0, 3), (3, 9), (9, 17), (17, KT)]

            def w_dma(ci):
                a, b = W_CHUNKS[ci]
                nc.sync.dma_start(w_sb[:, a:b, :], w_d[:, a:b, :])

            def wot_dma():
                nc.sync.dma_start(wot_sb[:], wot_d[:])

            # persistent activations
            p_act = ctx.enter_context(tc.tile_pool(name="p_act", bufs=1))
            qt = [p_act.tile([HD, T], bf16, tag=f"qt{h}", name=f"qt{h}")
                  for h in range(QH)]
            ktl = [p_act.tile([HD, T], bf16, tag=f"kt{g2}", name=f"kt{g2}")
                   for g2 in range(KVH)]
            vext = [p_act.tile([128, NTT, VCOL], bf16, tag=f"vx{g2}",
                               name=f"vx{g2}") for g2 in range(KVH)]
            for g2 in range(KVH):
                nc.vector.memset(vext[g2][:, :, HD:VCOL], 1.0)
            at_dense = [p_act.tile([128, 6, QCS], bf16, tag=f"at{qc}",
                                   name=f"at{qc}") for qc in range(QC)]
            sums_sb = p_act.tile([128, T], f32r, tag="sums_sb")
            rinv_sb = p_act.tile([128, T], bf16, tag="rinv_sb")
            # per-qc staging of rinv rows at partition 0 (reused across qc)
            rl8 = p_act.tile([1, QH, QCS], bf16, tag="rl8")
            amun = p_act.tile([128, QC * QH, QCS], bf16, tag="amun")

            # ---------------- pools ----------------
            # pool release must be LIFO: s2 (lives longest) is created
            # before s1 (phase-1 scope, closed first).
            s2 = ExitStack()
            sc_pool = s2.enter_context(
                tc.tile_pool(name="sc", bufs=2, space="PSUM"))
            po_pool = s2.enter_context(
                tc.tile_pool(name="po", bufs=1, space="PSUM"))
            bc2_pool = s2.enter_context(
                tc.tile_pool(name="bc2", bufs=1, space="PSUM"))
            pt_pool = s2.enter_context(tc.tile_pool(name="pt", bufs=3))
            am_pool = s2.enter_context(tc.tile_pool(name="am", bufs=2))

            s1 = ExitStack()
            xt_pool = s1.enter_context(tc.tile_pool(name="xt", bufs=2))
            ps1_pool = s1.enter_context(
                tc.tile_pool(name="ps1", bufs=1, space="PSUM"))
            pst_pool = s1.enter_context(
                tc.tile_pool(name="pst", bufs=1, space="PSUM"))
            w1_pool = s1.enter_context(tc.tile_pool(name="w1", bufs=2))

            # ---------------- phase 1 pieces ----------------
            psx_tiles = {}

            xt_tiles = {}

            def p1_xt(tt):
                xt_t = xt_pool.tile([128, KT, 128], bf16, tag="xt_t",
                                    name=f"xt{tt}")
                nc.sync.dma_start(xt_t[:], xt_d[:, tt])
                xt_tiles[tt] = xt_t

            def p1_mms(tt):
                xt_t = xt_tiles[tt]
                pss = []
                for c in range(NCH):
                    ps = ps1_pool.tile([128, CHW], f32, tag=f"ps{c}",
                                       name=f"ps{c}_{tt}")
                    pss.append(ps)
                for kt in range(KT):
                    for c in range(NCH):
                        nc.tensor.matmul(
                            pss[c][:],
                            xt_t[:, kt, :],
                            w_sb[:, kt, c * CHW:(c + 1) * CHW],
                            start=(kt == 0), stop=(kt == KT - 1))
                psx_tiles[tt] = pss

            roped_tiles = {}

            def p1_post(tt):
                pss = psx_tiles[tt]
                # copy psum -> sbuf bf16 (frees psum for next tt quickly)
                prj = w1_pool.tile([128, NCH, CHW], bf16, tag="prj",
                                   name=f"prj{tt}")
                for c in range(NCH):
                    nc.scalar.copy(prj[:, c, :], pss[c][:])
                # v heads: straight copy into vext (token-layout, no transpose)
                for g2 in range(KVH):
                    nc.gpsimd.tensor_copy(
                        vext[g2][:, tt, 0:HD],
                        prj[:, 2, HD * (KVH + g2):HD * (KVH + g2 + 1)])
                # rms: per-head sum of squares along free dim
                ssq = w1_pool.tile([128, RH], f32, tag="ssq", name=f"ssq{tt}")
                sqs = w1_pool.tile([128, HD], bf16, tag="sqs", name=f"sqs{tt}")
                for rh in range(RH):
                    c, off = divmod(rh * HD, CHW)
                    sl = prj[:, c, off:off + HD]
                    nc.vector.scalar_tensor_tensor(
                        sqs[:], sl, 1.0, sl,
                        op0=ALU.mult, op1=ALU.mult,
                        accum_out=ssq[:, rh:rh + 1])
                rms = w1_pool.tile([128, RH], f32, tag="rms", name=f"rms{tt}")
                nc.scalar.activation(rms[:], ssq[:], AF.Sqrt,
                                     bias=eps_t[:], scale=1.0 / HD)
                rinv = w1_pool.tile([128, RH], bf16, tag="rinv",
                                    name=f"rinv{tt}")
                nc.vector.reciprocal(rinv[:], rms[:])
                # rope (batched across heads per chunk)
                roped = w1_pool.tile([128, RH * HD], bf16, tag="roped",
                                     name=f"roped{tt}")
                tm0 = w1_pool.tile([128, CHW], bf16, tag="tm0", name=f"tm0{tt}")
                tm1 = w1_pool.tile([128, CHW], bf16, tag="tm1", name=f"tm1{tt}")
                tm2 = w1_pool.tile([128, CHW], bf16, tag="tm2", name=f"tm2{tt}")
                for c in range(NCH):
                    nh = 4 if c < 2 else 2
                    w96 = nh * HD
                    taba, tbe, tbo = ((tabaq, tabbqe, tabbqo) if c < 2
                                      else (tabak, tabbke, tabbko))
                    src = prj[:, c, 0:w96].rearrange("p (h d) -> p h d", h=nh)
                    rb = (rinv[:, c * 4:c * 4 + nh]
                          .unsqueeze(2).broadcast_to([128, nh, HD]))
                    t0 = tm0[:, 0:w96].rearrange("p (h d) -> p h d", h=nh)
                    nc.vector.tensor_mul(t0, src, rb)
                    ta = (taba[:, tt, :].unsqueeze(1)
                          .broadcast_to([128, nh, HD]))
                    t1 = tm1[:, 0:w96].rearrange("p (h d) -> p h d", h=nh)
                    nc.vector.tensor_mul(t1, t0, ta)
                    t0p = tm0[:, 0:w96].rearrange("p (h i e) -> p h i e",
                                                  h=nh, e=2)
                    t2p = tm2[:, 0:w96].rearrange("p (h i e) -> p h i e",
                                                  h=nh, e=2)
                    tbe_b = (tbe[:, tt, :].unsqueeze(1)
                             .broadcast_to([128, nh, HD // 2]))
                    tbo_b = (tbo[:, tt, :].unsqueeze(1)
                             .broadcast_to([128, nh, HD // 2]))
                    nc.vector.tensor_mul(t2p[:, :, :, 0], t0p[:, :, :, 1],
                                         tbe_b)
                    nc.vector.tensor_mul(t2p[:, :, :, 1], t0p[:, :, :, 0],
                                         tbo_b)
                    dst = roped[:, c * CHW:c * CHW + w96]
                    nc.vector.tensor_add(dst, tm1[:, 0:w96], tm2[:, 0:w96])
                roped_tiles[tt] = roped

            def p1_transposes(tt):
                roped = roped_tiles[tt]
                for rh in range(RH):
                    pst = pst_pool.tile([128, 128], bf16, tag="pst",
                                        name=f"pst{tt}_{rh}")
                    nc.tensor.transpose(
                        pst[0:HD, :],
                        roped[:, rh * HD:(rh + 1) * HD],
                        ident[:])
                    dst = qt[rh] if rh < QH else ktl[rh - QH]
                    nc.scalar.copy(dst[:, tt * 128:(tt + 1) * 128],
                                   pst[0:HD, :])

            # ---------------- phase 2 pieces ----------------
            def p2_head(h, qc, pools=None):
                g2 = h // (QH // KVH)
                qsl = slice(qc * QCS, (qc + 1) * QCS)
                nkt = (qc + 1) * QCS // 128
                po = po_pool.tile([128, QCS], f32, tag="po",
                                  name=f"po{qc}_{h}")
                poa = po[0:HD + h + 1, :]
                pts = {}
                scp = pools or [sc_pool]

                def sc_mm(kt2):
                    sc = scp[kt2 % len(scp)].tile(
                        [128, QCS], f32, tag="sc",
                        name=f"sc{qc}_{h}_{kt2}")
                    nc.tensor.matmul(
                        sc[:],
                        ktl[g2][:, kt2 * 128:(kt2 + 1) * 128],
                        qt[h][:, qsl],
                        start=True, stop=True)
                    return sc

                def pt_make(kt2, sc):
                    pt = pt_pool.tile([128, QCS], bf16, tag="pt",
                                      name=f"pt{qc}_{h}_{kt2}")
                    nc.scalar.activation(pt[:], sc[:], AF.Exp, scale=SCALE)
                    if kt2 >= qc * (QCS // 128):
                        nc.gpsimd.affine_select(
                            pt[:], pt[:], pattern=[[1, QCS]],
                            compare_op=ALU.is_ge,
                            fill=0.0,
                            base=qc * QCS - kt2 * 128,
                            channel_multiplier=-1)
                    pts[kt2] = pt

                def pv_mm(kt2):
                    nc.tensor.matmul(
                        poa, vext[g2][:, kt2, 0:HD + h + 1],
                        pts[kt2][:],
                        start=(kt2 == 0), stop=(kt2 == nkt - 1))
                    pts[kt2] = None

                # software-pipelined: keep 2 sc tiles in flight
                sc0 = sc_mm(0)
                pt_make(0, sc0)
                for kt2 in range(1, nkt):
                    sc_n = sc_mm(kt2)
                    pv_mm(kt2 - 1)
                    pt_make(kt2, sc_n)
                pv_mm(nkt - 1)
                # denominator: po rows 96..96+h all hold this head's sum.
                # Engine partition starts must be 32-aligned, so copy the
                # whole [96:97+h] block; heads are processed in DESCENDING
                # order so each later (smaller) copy leaves row 96+h' of
                # earlier heads h' > h intact.
                nc.scalar.copy(sums_sb[HD:HD + h + 1, qsl],
                               po[HD:HD + h + 1, :])
                nc.vector.tensor_copy(amun[0:HD, qc * QH + h, :], po[0:HD, :])

            def p2_recip(qc, lo, hi):
                # engine partition starts must be 32-aligned: always start
                # at 96.  For the "hi" half (lo=4) the low rows are junk at
                # this point and get recomputed by the later lo pass.
                qsl = slice(qc * QCS, (qc + 1) * QCS)
                nc.vector.reciprocal(rinv_sb[HD:HD + hi, qsl],
                                     sums_sb[HD:HD + hi, qsl])
                # matmul lhsT/rhs base partition must be in {0,32,64}:
                # shift each head's rinv row down to partition 0 via DMA
                for h in range(lo, hi):
                    nc.sync.dma_start(rl8[0:1, h, :],
                                      rinv_sb[HD + h:HD + h + 1, qsl])

            def p2_norm(h, qc):
                qsl = slice(qc * QCS, (qc + 1) * QCS)
                bc2 = bc2_pool.tile([128, QCS], f32, tag="bc2",
                                    name=f"bc2{qc}_{h}")
                nc.tensor.matmul(
                    bc2[0:HD, :],
                    ones_t[0:1, :],
                    rl8[0:1, h, :],
                    start=True, stop=True)
                am = am_pool.tile([HD, QCS], bf16, tag="am",
                                  name=f"am{qc}_{h}")
                nc.vector.tensor_mul(am[:], amun[0:HD, qc * QH + h, :],
                                     bc2[0:HD, :])
                # scatter into dense [768 = 6x128] layout (partition shift
                # is only legal via DMA)
                r0 = h * HD
                t0, off = divmod(r0, 128)
                n1 = min(128 - off, HD)
                nc.sync.dma_start(at_dense[qc][off:off + n1, t0, :],
                                  am[0:n1, :])
                if n1 < HD:
                    nc.sync.dma_start(
                        at_dense[qc][0:HD - n1, t0 + 1, :],
                        am[n1:HD, :])

            # ---------------- phase 3 pieces ----------------
            s3 = ExitStack()
            ps3_pool = None
            scx_pool = None
            ob_pool = None

            def p3_open():
                nonlocal ps3_pool, scx_pool, ob_pool
                ps3_pool = s3.enter_context(
                    tc.tile_pool(name="ps3", bufs=2, space="PSUM"))
                scx_pool = s3.enter_context(
                    tc.tile_pool(name="scx", bufs=2, space="PSUM"))
                ob_pool = s3.enter_context(tc.tile_pool(name="ob", bufs=1))

            def p3_iblock(i):
                qc = i // (QCS // 128)
                isl = slice((i % (QCS // 128)) * 128,
                            (i % (QCS // 128)) * 128 + 128)
                ob = ob_pool.tile([128, NJ, 512], bf16, tag="ob",
                                  name=f"ob{i}")
                for j in range(NJ):
                    ps3 = ps3_pool.tile([128, 512], f32, tag="ps3",
                                        name=f"ps3_{i}_{j}")
                    for t6 in range(6):
                        nc.tensor.matmul(
                            ps3[:], at_dense[qc][:, t6, isl],
                            wot_sb[:, t6, j * 512:(j + 1) * 512],
                            start=(t6 == 0), stop=(t6 == 5))
                    # alternate the psum->sbuf copies between DVE and ACT
                    # so neither queue gates the ps3 double-buffer rotation
                    if j % 2 == 0:
                        nc.vector.tensor_copy(ob[:, j, :], ps3[:])
                    else:
                        nc.scalar.copy(ob[:, j, :], ps3[:])
                nc.sync.dma_start(
                    out_d[i * 128:(i + 1) * 128, :],
                    ob[:].rearrange("p a b -> p (a b)"))

            # ---------------- emission schedule ----------------
            with s1:
                # DMA issue order: xt(0) first (small, unblocks first mms),
                # then W chunks (deps require emission before the mms that
                # read them), tables, then xt(tt) prefetched per iteration.
                p1_xt(0)
                for ci in range(len(W_CHUNKS)):
                    w_dma(ci)
                tab_dmas()
                for tt in range(4):
                    p1_xt(tt + 1)
                    p1_mms(tt)
                    if tt == 2:
                        wot_dma()
                    if tt > 0:
                        p1_transposes(tt - 1)
                    p1_post(tt)
                for tt in range(4, NTT):
                    if tt + 1 < NTT:
                        p1_xt(tt + 1)
                    p1_mms(tt)
                    p1_transposes(tt - 1)
                    p1_post(tt)
                    p2_head(QH - 1 - (tt - 4), 0)
                p1_transposes(NTT - 1)
            # phase-1 psum freed; open phase-3 pools (+ extra score bufs)
            p3_open()
            with s3:
                AB = [sc_pool, scx_pool]
                # interleave the qc0 tail with the high qc1 heads (both
                # descending per qc for the sums-copy clobber rule)
                p2_head(3, 0, AB)
                p2_head(7, 1, AB)
                p2_head(2, 0, AB)
                p2_head(6, 1, AB)
                p2_head(1, 0, AB)
                p2_head(5, 1, AB)
                p2_head(0, 0, AB)
                p2_recip(0, 0, QH)
                p2_head(4, 1, AB)
                for h in (7, 6, 5, 4):
                    p2_norm(h, 0)
                p2_head(3, 1, AB)
                for h in (3, 2, 1, 0):
                    p2_norm(h, 0)
                p2_head(2, 1, AB)
                p2_recip(1, 4, QH)
                for h in (7, 6, 5, 4):
                    p2_norm(h, 1)
                p2_head(1, 1, AB)
                p3_iblock(0)
                p2_head(0, 1, AB)
                p3_iblock(1)
                p2_recip(1, 0, 4)
                for h in (3, 2, 1, 0):
                    p2_norm(h, 1)
                p3_iblock(2)
                p3_iblock(3)
                for i in range(4, NTT):
                    p3_iblock(i)
            s2.close()

    nc.compile()
    return nc


def get_nc():
    if "nc" not in _BUILD_CACHE:
        _BUILD_CACHE["nc"] = _build_nc()
    return _BUILD_CACHE["nc"]


def prepare_in_maps(x, wq, wk, wv, wo, q_norm_w, k_norm_w, cos, sin):
    import ml_dtypes
    bf = ml_dtypes.bfloat16

    x = np.asarray(x, np.float32)
    wq = np.asarray(wq, np.float32)
    wk = np.asarray(wk, np.float32)
    wv = np.asarray(wv, np.float32)
    wo = np.asarray(wo, np.float32)
    cos = np.asarray(cos, np.float32)
    sin = np.asarray(sin, np.float32)
    qw = np.asarray(q_norm_w, np.float32)
    kw = np.asarray(k_norm_w, np.float32)

    # rope tables in token-tile layout [128, NTT, ...]
    def tok_tiles(a):  # (T, F) -> (128, NTT, F)
        F = a.shape[1]
        return np.ascontiguousarray(
            a.reshape(NTT, 128, F).transpose(1, 0, 2)).astype(bf)

    tabaq = tok_tiles(cos * qw[None, :])
    tabak = tok_tiles(cos * kw[None, :])
    tabbqe = tok_tiles(-sin[:, 0::2] * qw[None, 1::2])
    tabbqo = tok_tiles(sin[:, 1::2] * qw[None, 0::2])
    tabbke = tok_tiles(-sin[:, 0::2] * kw[None, 1::2])
    tabbko = tok_tiles(sin[:, 1::2] * kw[None, 0::2])

    # x: [128, NTT, KT, 128] per batch (contraction tiles on partitions)
    xts = []
    for b in range(B):
        xT = x[b].T  # (D, T)
        # (KT,128,T) -> partitions first, then token tiles contiguous
        t1 = xT.reshape(KT, 128, NTT, 128)
        xts.append(np.ascontiguousarray(
            t1.transpose(1, 2, 0, 3)).astype(bf))

    in_maps = []
    wcache = {}
    for c in range(NCORES):
        b, g = divmod(c, G)
        if g not in wcache:
            # W columns: q heads g*8..g*8+7, then k0,k1, v0,v1 (96 each)
            cols = [wq[(g * QH + i) * HD:(g * QH + i + 1) * HD]
                    for i in range(QH)]
            cols += [wk[(g * KVH + i) * HD:(g * KVH + i + 1) * HD]
                     for i in range(KVH)]
            cols += [wv[(g * KVH + i) * HD:(g * KVH + i + 1) * HD]
                     for i in range(KVH)]
            wall = np.concatenate(cols, axis=0).T  # (D, 1152)
            w_t = np.ascontiguousarray(
                wall.reshape(KT, 128, PRJ).transpose(1, 0, 2)).astype(bf)
            # wot: rows = packed [768] head dims, cols = D
            wo_sh = wo[:, g * QH * HD:(g + 1) * QH * HD]  # (D, 768)
            wot = np.ascontiguousarray(
                wo_sh.T.reshape(6, 128, D).transpose(1, 0, 2)).astype(bf)
            wcache[g] = (w_t, wot)
        w_t, wot = wcache[g]
        in_maps.append({
            "xt": xts[b], "w": w_t, "wot": wot,
            "tabaq": tabaq, "tabbqe": tabbqe, "tabbqo": tabbqo,
            "tabak": tabak, "tabbke": tabbke, "tabbko": tabbko,
        })
    return in_maps


def kernel(**inputs):
    from concourse import bass_utils

    nc = get_nc()
    in_maps = prepare_in_maps(
        inputs["x"], inputs["wq"], inputs["wk"], inputs["wv"], inputs["wo"],
        inputs["q_norm_w"], inputs["k_norm_w"], inputs["cos"], inputs["sin"])
    trace = bool(int(os.environ.get("BASS_KERNEL_TRACE", "0")))
    res = bass_utils.run_bass_kernel_spmd(
        nc, in_maps, core_ids=list(range(NCORES)), trace=trace)
    _BUILD_CACHE["last_result"] = res
    partials = [np.asarray(r["out"], np.float32) for r in res.results]
    out = np.empty((B, T, D), np.float32)
    for b in range(B):
        out[b] = np.sum(np.stack(partials[b * G:(b + 1) * G]), axis=0,
                        dtype=np.float64).astype(np.float32)
    return out


# revision 38
# speedup vs baseline: 1.8136x; 1.0905x over previous
"""Grouped-Query Attention block (RMSNorm + RoPE + causal GQA + o_proj) on 8 trn2 NeuronCores.

Sharding: data-parallel over batch (2) x tensor-parallel over kv-head groups (4).
Core c = b*4 + g handles batch b, kv heads {2g, 2g+1}, q heads {8g..8g+7}.
Each core computes a partial o_proj output (T, D) over its 768 head-dims;
host sums the 4 group partials per batch.

v2 design (vs v1):
  * Phase 1 projections in token-on-partition layout: x tile is the
    stationary operand, all 12 head outputs (8q+2k+2v = 1152 dims) are the
    moving free dim in 3 dense 384-wide chunks -> 25% fewer PE cycles than
    per-head padded outputs, and RMSNorm/RoPE become cheap free-dim ops
    (per-token scalars live on partitions: tensor_scalar / [128,10]
    reciprocals instead of [1,512] single-lane reciprocals + PE broadcasts).
  * bf16 everywhere on the matmul path (same PE rate as fp32r at these
    free sizes, half the DMA/SBUF traffic).
  * q/k transposed back to [head_dim, T] via PE transpose-mode (96x128).
  * v needs no transpose at all in this layout (it was 16 PE transposes in v1).
  * Attention: scores transposed (k on partitions) with softmax k-sum folded
    into PV via per-head ones-columns appended to V at column 96+h, so each
    head's denominator lands on its own partition -> batched [8,512]
    reciprocal instead of 16 serial [1,512] ones (120us of DVE in v1).
  * Causal mask applied structurally (skip above-diagonal k-tiles, gpsimd
    affine_select on the rest) - same as v1.
  * Phase 3 o_proj over a densely packed [768 = 6x128, T] activation
    (head boundaries straddle tiles; packing via SBUF-SBUF DMA) -> 6
    contraction tiles instead of 8 per output tile: 25% fewer PE cycles.
  * Emission is software-pipelined so the PE queue never head-blocks on
    the ACT/DVE/gpsimd post-processing chains: transposes for token-tile
    tt are emitted after the projections of tt+1; attention q-chunk 0 is
    interleaved into the phase-1 tail; o_proj i-blocks are interleaved
    into attention q-chunk 1.
"""

import os
import sys

import numpy as np

sys.path.insert(0, "/opt/trn_rl_repo")

B, T, D = 2, 1024, 3072
NH, NKV, HD = 32, 8, 96
G = 4                 # tensor-parallel groups
QH = NH // G          # q heads per core (8)
KVH = NKV // G        # kv heads per core (2)
NCORES = 8
EPS = 1e-6
SCALE = 1.0 / float(np.sqrt(HD))
KT = D // 128          # 24 contraction tiles over d_model
NTT = T // 128         # 8 token tiles
QC = 2                 # q chunks in phase 2
QCS = T // QC          # 512
NJ = D // 512          # 6 o_proj output column chunks
RH = QH + KVH          # 10 rope heads (8 q + 2 k)
PRJ = QH * HD + KVH * HD * 2   # 1152 projected dims per core
NCH = 3                # projection chunks
CHW = PRJ // NCH       # 384
VCOL = HD + QH         # 104: v columns + per-head ones columns

_BUILD_CACHE = {}


def _build_nc():
    from contextlib import ExitStack
    from concourse import bacc, tile, mybir

    f32 = mybir.dt.float32
    f32r = mybir.dt.float32r
    bf16 = mybir.dt.bfloat16
    AF = mybir.ActivationFunctionType
    ALU = mybir.AluOpType

    nc = bacc.Bacc("TRN2", target_bir_lowering=False, debug=False,
                   num_devices=NCORES)

    xt_d = nc.dram_tensor("xt", (128, NTT, KT, 128), bf16, kind="ExternalInput").ap()
    w_d = nc.dram_tensor("w", (128, KT, PRJ), bf16, kind="ExternalInput").ap()
    wot_d = nc.dram_tensor("wot", (128, 6, D), bf16, kind="ExternalInput").ap()
    tabaq_d = nc.dram_tensor("tabaq", (128, NTT, HD), bf16, kind="ExternalInput").ap()
    tabbqe_d = nc.dram_tensor("tabbqe", (128, NTT, HD // 2), bf16, kind="ExternalInput").ap()
    tabbqo_d = nc.dram_tensor("tabbqo", (128, NTT, HD // 2), bf16, kind="ExternalInput").ap()
    tabak_d = nc.dram_tensor("tabak", (128, NTT, HD), bf16, kind="ExternalInput").ap()
    tabbke_d = nc.dram_tensor("tabbke", (128, NTT, HD // 2), bf16, kind="ExternalInput").ap()
    tabbko_d = nc.dram_tensor("tabbko", (128, NTT, HD // 2), bf16, kind="ExternalInput").ap()
    out_d = nc.dram_tensor("out", (T, D), bf16, kind="ExternalOutput").ap()

    with tile.TileContext(nc) as tc:
        with nc.allow_low_precision(reason="bf16 matmul path, fp32 accum"), \
             ExitStack() as ctx:
            const = ctx.enter_context(tc.tile_pool(name="const", bufs=1))

            ident = const.tile([128, 128], bf16, tag="ident")
            from concourse.masks import make_identity
            make_identity(nc, ident[:])
            eps_t = const.tile([128, 1], f32, tag="eps")
            nc.vector.memset(eps_t[:], EPS)
            ones_t = const.tile([128, HD], bf16, tag="ones_t")
            nc.vector.memset(ones_t[:], 1.0)

            # rope/norm tables (token-tile layout); DMA issues deferred
            tabs = ctx.enter_context(tc.tile_pool(name="tabs", bufs=1))
            tabaq = tabs.tile([128, NTT, HD], bf16, tag="tabaq")
            tabbqe = tabs.tile([128, NTT, HD // 2], bf16, tag="tabbqe")
            tabbqo = tabs.tile([128, NTT, HD // 2], bf16, tag="tabbqo")
            tabak = tabs.tile([128, NTT, HD], bf16, tag="tabak")
            tabbke = tabs.tile([128, NTT, HD // 2], bf16, tag="tabbke")
            tabbko = tabs.tile([128, NTT, HD // 2], bf16, tag="tabbko")

            def tab_dmas():
                for t, td in ((tabaq, tabaq_d), (tabbqe, tabbqe_d),
                              (tabbqo, tabbqo_d), (tabak, tabak_d),
                              (tabbke, tabbke_d), (tabbko, tabbko_d)):
                    nc.sync.dma_start(t[:], td[:])

            # weights resident in SBUF.  DMA issue order matters: the W
            # chunks are issued interleaved with the first token tiles so
            # the first projection matmuls are not stuck behind 12MB of
            # weight traffic; wot (only needed by o_proj, >200us in) is
            # issued mid-phase-1.
            wpool = ctx.enter_context(tc.tile_pool(name="wpool", bufs=1))
            w_sb = wpool.tile([128, KT, PRJ], bf16, tag="w_sb")
            wot_sb = wpool.tile([128, 6, D], bf16, tag="wot_sb")
            W_CHUNKS = [(0, 3), (3, 9), (9, 17), (17, KT)]

            def w_dma(ci):
                a, b = W_CHUNKS[ci]
                nc.sync.dma_start(w_sb[:, a:b, :], w_d[:, a:b, :])

            def wot_dma():
                nc.sync.dma_start(wot_sb[:], wot_d[:])

            # persistent activations
            p_act = ctx.enter_context(tc.tile_pool(name="p_act", bufs=1))
            qt = [p_act.tile([HD, T], bf16, tag=f"qt{h}", name=f"qt{h}")
                  for h in range(QH)]
            ktl = [p_act.tile([HD, T], bf16, tag=f"kt{g2}", name=f"kt{g2}")
                   for g2 in range(KVH)]
            vext = [p_act.tile([128, NTT, VCOL], bf16, tag=f"vx{g2}",
                               name=f"vx{g2}") for g2 in range(KVH)]
            for g2 in range(KVH):
                nc.vector.memset(vext[g2][:, :, HD:VCOL], 1.0)
            at_dense = [p_act.tile([128, 6, QCS], bf16, tag=f"at{qc}",
                                   name=f"at{qc}") for qc in range(QC)]
            sums_sb = p_act.tile([128, T], f32r, tag="sums_sb")
            rinv_sb = p_act.tile([128, T], bf16, tag="rinv_sb")
            # per-qc staging of rinv rows at partition 0 (reused across qc)
            rl8 = p_act.tile([1, QH, QCS], bf16, tag="rl8")
            amun = p_act.tile([128, QC * QH, QCS], bf16, tag="amun")

            # ---------------- pools ----------------
            # pool release must be LIFO: s2 (lives longest) is created
            # before s1 (phase-1 scope, closed first).
            s2 = ExitStack()
            sc_pool = s2.enter_context(
                tc.tile_pool(name="sc", bufs=2, space="PSUM"))
            po_pool = s2.enter_context(
                tc.tile_pool(name="po", bufs=1, space="PSUM"))
            bc2_pool = s2.enter_context(
                tc.tile_pool(name="bc2", bufs=1, space="PSUM"))
            pt_pool = s2.enter_context(tc.tile_pool(name="pt", bufs=3))
            am_pool = s2.enter_context(tc.tile_pool(name="am", bufs=2))

            s1 = ExitStack()
            xt_pool = s1.enter_context(tc.tile_pool(name="xt", bufs=2))
            ps1_pool = s1.enter_context(
                tc.tile_pool(name="ps1", bufs=1, space="PSUM"))
            pst_pool = s1.enter_context(
                tc.tile_pool(name="pst", bufs=1, space="PSUM"))
            w1_pool = s1.enter_context(tc.tile_pool(name="w1", bufs=2))

            # ---------------- phase 1 pieces ----------------
            psx_tiles = {}

            xt_tiles = {}

            def p1_xt(tt):
                xt_t = xt_pool.tile([128, KT, 128], bf16, tag="xt_t",
                                    name=f"xt{tt}")
                nc.sync.dma_start(xt_t[:], xt_d[:, tt])
                xt_tiles[tt] = xt_t

            def p1_mms(tt):
                xt_t = xt_tiles[tt]
                pss = []
                for c in range(NCH):
                    ps = ps1_pool.tile([128, CHW], f32, tag=f"ps{c}",
                                       name=f"ps{c}_{tt}")
                    pss.append(ps)
                for kt in range(KT):
                    for c in range(NCH):
                        nc.tensor.matmul(
                            pss[c][:],
                            xt_t[:, kt, :],
                            w_sb[:, kt, c * CHW:(c + 1) * CHW],
                            start=(kt == 0), stop=(kt == KT - 1))
                psx_tiles[tt] = pss

            roped_tiles = {}

            def p1_post(tt):
                pss = psx_tiles[tt]
                # copy psum -> sbuf bf16 (frees psum for next tt quickly)
                prj = w1_pool.tile([128, NCH, CHW], bf16, tag="prj",
                                   name=f"prj{tt}")
                for c in range(NCH):
                    nc.scalar.copy(prj[:, c, :], pss[c][:])
                # v heads: straight copy into vext (token-layout, no transpose)
                for g2 in range(KVH):
                    nc.gpsimd.tensor_copy(
                        vext[g2][:, tt, 0:HD],
                        prj[:, 2, HD * (KVH + g2):HD * (KVH + g2 + 1)])
                # rms: per-head sum of squares along free dim
                ssq = w1_pool.tile([128, RH], f32, tag="ssq", name=f"ssq{tt}")
                sqs = w1_pool.tile([128, HD], bf16, tag="sqs", name=f"sqs{tt}")
                for rh in range(RH):
                    c, off = divmod(rh * HD, CHW)
                    sl = prj[:, c, off:off + HD]
                    nc.vector.scalar_tensor_tensor(
                        sqs[:], sl, 1.0, sl,
                        op0=ALU.mult, op1=ALU.mult,
                        accum_out=ssq[:, rh:rh + 1])
                rms = w1_pool.tile([128, RH], f32, tag="rms", name=f"rms{tt}")
                nc.scalar.activation(rms[:], ssq[:], AF.Sqrt,
                                     bias=eps_t[:], scale=1.0 / HD)
                rinv = w1_pool.tile([128, RH], bf16, tag="rinv",
                                    name=f"rinv{tt}")
                nc.vector.reciprocal(rinv[:], rms[:])
                # rope (batched across heads per chunk)
                roped = w1_pool.tile([128, RH * HD], bf16, tag="roped",
                                     name=f"roped{tt}")
                tm0 = w1_pool.tile([128, CHW], bf16, tag="tm0", name=f"tm0{tt}")
                tm1 = w1_pool.tile([128, CHW], bf16, tag="tm1", name=f"tm1{tt}")
                tm2 = w1_pool.tile([128, CHW], bf16, tag="tm2", name=f"tm2{tt}")
                for c in range(NCH):
                    nh = 4 if c < 2 else 2
                    w96 = nh * HD
                    taba, tbe, tbo = ((tabaq, tabbqe, tabbqo) if c < 2
                                      else (tabak, tabbke, tabbko))
                    src = prj[:, c, 0:w96].rearrange("p (h d) -> p h d", h=nh)
                    rb = (rinv[:, c * 4:c * 4 + nh]
                          .unsqueeze(2).broadcast_to([128, nh, HD]))
                    t0 = tm0[:, 0:w96].rearrange("p (h d) -> p h d", h=nh)
                    nc.vector.tensor_mul(t0, src, rb)
                    ta = (taba[:, tt, :].unsqueeze(1)
                          .broadcast_to([128, nh, HD]))
                    t1 = tm1[:, 0:w96].rearrange("p (h d) -> p h d", h=nh)
                    nc.vector.tensor_mul(t1, t0, ta)
                    t0p = tm0[:, 0:w96].rearrange("p (h i e) -> p h i e",
                                                  h=nh, e=2)
                    t2p = tm2[:, 0:w96].rearrange("p (h i e) -> p h i e",
                                                  h=nh, e=2)
                    tbe_b = (tbe[:, tt, :].unsqueeze(1)
                             .broadcast_to([128, nh, HD // 2]))
                    tbo_b = (tbo[:, tt, :].unsqueeze(1)
                             .broadcast_to([128, nh, HD // 2]))
                    nc.vector.tensor_mul(t2p[:, :, :, 0], t0p[:, :, :, 1],
                                         tbe_b)
                    nc.vector.tensor_mul(t2p[:, :, :, 1], t0p[:, :, :, 0],
                                         tbo_b)
                    dst = roped[:, c * CHW:c * CHW + w96]
                    nc.vector.tensor_add(dst, tm1[:, 0:w96], tm2[:, 0:w96])
                roped_tiles[tt] = roped

            def p1_transposes(tt):
                roped = roped_tiles[tt]
                for rh in range(RH):
                    pst = pst_pool.tile([128, 128], bf16, tag="pst",
                                        name=f"pst{tt}_{rh}")
                    nc.tensor.transpose(
                        pst[0:HD, :],
                        roped[:, rh * HD:(rh + 1) * HD],
                        ident[:])
                    dst = qt[rh] if rh < QH else ktl[rh - QH]
                    nc.scalar.copy(dst[:, tt * 128:(tt + 1) * 128],
                                   pst[0:HD, :])

            # ---------------- phase 2 pieces ----------------
            # score tiles are trimmed to the causal region: for k-tile kt2
            # only q >= kt2*128 can attend, so the q-range of every score/
            # exp/PV op starts at max(qc*QCS, kt2*128).  The masked
            # diagonal triangle is always the first 128 columns of the
            # trimmed tile (base 0 affine_select).
            def p2_head(h, qc, pools=None, pop=None):
                g2 = h // (QH // KVH)
                nkt = (qc + 1) * QCS // 128
                po = (pop or po_pool).tile([128, QCS], f32, tag="po",
                                           name=f"po{qc}_{h}")
                pts = {}
                scp = pools or [sc_pool]

                def qoff_of(kt2):
                    return max(0, kt2 * 128 - qc * QCS)

                def sc_mm(kt2):
                    qoff = qoff_of(kt2)
                    tw = QCS - qoff
                    sc = scp[kt2 % len(scp)].tile(
                        [128, QCS], f32, tag="sc",
                        name=f"sc{qc}_{h}_{kt2}")
                    nc.tensor.matmul(
                        sc[:, 0:tw],
                        ktl[g2][:, kt2 * 128:(kt2 + 1) * 128],
                        qt[h][:, qc * QCS + qoff:(qc + 1) * QCS],
                        start=True, stop=True)
                    return sc

                def pt_make(kt2, sc):
                    qoff = qoff_of(kt2)
                    tw = QCS - qoff
                    pt = pt_pool.tile([128, QCS], bf16, tag="pt",
                                      name=f"pt{qc}_{h}_{kt2}")
                    nc.scalar.activation(pt[:, 0:tw], sc[:, 0:tw], AF.Exp,
                                         scale=SCALE)
                    if kt2 * 128 >= qc * QCS:
                        nc.gpsimd.affine_select(
                            pt[:, 0:128], pt[:, 0:128], pattern=[[1, 128]],
                            compare_op=ALU.is_ge,
                            fill=0.0,
                            base=0,
                            channel_multiplier=-1)
                    pts[kt2] = pt

                def pv_mm(kt2):
                    qoff = qoff_of(kt2)
                    tw = QCS - qoff
                    nc.tensor.matmul(
                        po[0:HD + h + 1, qoff:QCS],
                        vext[g2][:, kt2, 0:HD + h + 1],
                        pts[kt2][:, 0:tw],
                        start=(kt2 == 0), stop=(kt2 == nkt - 1),
                        skip_group_check=True)
                    pts[kt2] = None

                # software-pipelined: keep 2 sc tiles in flight
                sc0 = sc_mm(0)
                pt_make(0, sc0)
                for kt2 in range(1, nkt):
                    sc_n = sc_mm(kt2)
                    pv_mm(kt2 - 1)
                    pt_make(kt2, sc_n)
                pv_mm(nkt - 1)
                # denominator: po rows 96..96+h all hold this head's sum.
                # Engine partition starts must be 32-aligned, so copy the
                # whole [96:97+h] block; heads are processed in DESCENDING
                # order so each later (smaller) copy leaves row 96+h' of
                # earlier heads h' > h intact.
                qsl = slice(qc * QCS, (qc + 1) * QCS)
                nc.scalar.copy(sums_sb[HD:HD + h + 1, qsl],
                               po[HD:HD + h + 1, :])
                nc.vector.tensor_copy(amun[0:HD, qc * QH + h, :], po[0:HD, :])

            def p2_recip(qc, lo, hi):
                # engine partition starts must be 32-aligned: always start
                # at 96.  For the "hi" half (lo=4) the low rows are junk at
                # this point and get recomputed by the later lo pass.
                qsl = slice(qc * QCS, (qc + 1) * QCS)
                nc.vector.reciprocal(rinv_sb[HD:HD + hi, qsl],
                                     sums_sb[HD:HD + hi, qsl])
                # matmul lhsT/rhs base partition must be in {0,32,64}:
                # shift each head's rinv row down to partition 0 via DMA
                for h in range(lo, hi):
                    nc.sync.dma_start(rl8[0:1, h, :],
                                      rinv_sb[HD + h:HD + h + 1, qsl])

            def p2_norm(h, qc):
                qsl = slice(qc * QCS, (qc + 1) * QCS)
                bc2 = bc2_pool.tile([128, QCS], f32, tag="bc2",
                                    name=f"bc2{qc}_{h}")
                nc.tensor.matmul(
                    bc2[0:HD, :],
                    ones_t[0:1, :],
                    rl8[0:1, h, :],
                    start=True, stop=True)
                am = am_pool.tile([HD, QCS], bf16, tag="am",
                                  name=f"am{qc}_{h}")
                nc.vector.tensor_mul(am[:], amun[0:HD, qc * QH + h, :],
                                     bc2[0:HD, :])
                # scatter into dense [768 = 6x128] layout (partition shift
                # is only legal via DMA)
                r0 = h * HD
                t0, off = divmod(r0, 128)
                n1 = min(128 - off, HD)
                nc.sync.dma_start(at_dense[qc][off:off + n1, t0, :],
                                  am[0:n1, :])
                if n1 < HD:
                    nc.sync.dma_start(
                        at_dense[qc][0:HD - n1, t0 + 1, :],
                        am[n1:HD, :])

            # ---------------- phase 3 pieces ----------------
            s3 = ExitStack()
            ps3_pool = None
            scx_pool = None
            ob_pool = None

            pox_pool = None

            def p3_open():
                nonlocal ps3_pool, scx_pool, ob_pool, pox_pool
                ps3_pool = s3.enter_context(
                    tc.tile_pool(name="ps3", bufs=2, space="PSUM"))
                scx_pool = s3.enter_context(
                    tc.tile_pool(name="scx", bufs=1, space="PSUM"))
                pox_pool = s3.enter_context(
                    tc.tile_pool(name="pox", bufs=1, space="PSUM"))
                ob_pool = s3.enter_context(tc.tile_pool(name="ob", bufs=1))

            def p3_iblock(i):
                qc = i // (QCS // 128)
                isl = slice((i % (QCS // 128)) * 128,
                            (i % (QCS // 128)) * 128 + 128)
                ob = ob_pool.tile([128, NJ, 512], bf16, tag="ob",
                                  name=f"ob{i}")
                for j in range(NJ):
                    ps3 = ps3_pool.tile([128, 512], f32, tag="ps3",
                                        name=f"ps3_{i}_{j}")
                    for t6 in range(6):
                        nc.tensor.matmul(
                            ps3[:], at_dense[qc][:, t6, isl],
                            wot_sb[:, t6, j * 512:(j + 1) * 512],
                            start=(t6 == 0), stop=(t6 == 5))
                    # alternate the psum->sbuf copies between DVE and ACT
                    # so neither queue gates the ps3 double-buffer rotation
                    if j % 2 == 0:
                        nc.vector.tensor_copy(ob[:, j, :], ps3[:])
                    else:
                        nc.scalar.copy(ob[:, j, :], ps3[:])
                    # store per j-chunk so the single ob buffer never gates
                    # the next i-block behind one big 768KB DMA
                    nc.sync.dma_start(
                        out_d[i * 128:(i + 1) * 128, j * 512:(j + 1) * 512],
                        ob[:, j, :])

            # ---------------- emission schedule ----------------
            with s1:
                # DMA issue order: xt(0) first (small, unblocks first mms),
                # then W chunks (deps require emission before the mms that
                # read them), tables, then xt(tt) prefetched per iteration.
                p1_xt(0)
                for ci in range(len(W_CHUNKS)):
                    w_dma(ci)
                tab_dmas()
                for tt in range(4):
                    p1_xt(tt + 1)
                    p1_mms(tt)
                    if tt == 2:
                        wot_dma()
                    if tt > 0:
                        p1_transposes(tt - 1)
                    p1_post(tt)
                for tt in range(4, NTT):
                    if tt + 1 < NTT:
                        p1_xt(tt + 1)
                    p1_mms(tt)
                    p1_transposes(tt - 1)
                    p1_post(tt)
                    p2_head(QH - 1 - (tt - 4), 0)
                p1_transposes(NTT - 1)
            # phase-1 psum freed; open phase-3 pools (+ extra score bufs)
            p3_open()
            with s3:
                AB = [sc_pool, scx_pool]
                pos = [po_pool, pox_pool]
                # interleave the qc0 tail with the high qc1 heads (both
                # descending per qc for the sums-copy clobber rule);
                # alternate po pools so one head's drain never gates the
                # next head's PV accumulation
                seq = [(3, 0), (7, 1), (2, 0), (6, 1), (1, 0), (5, 1),
                       (0, 0)]
                for n, (h, qc) in enumerate(seq):
                    p2_head(h, qc, AB, pos[n % 2])
                p2_recip(0, 0, QH)
                p2_head(4, 1, AB, pos[1])
                for h in (7, 6, 5, 4):
                    p2_norm(h, 0)
                p2_head(3, 1, AB, pos[0])
                for h in (3, 2, 1, 0):
                    p2_norm(h, 0)
                p2_head(2, 1, AB, pos[1])
                p2_recip(1, 4, QH)
                p2_head(1, 1, AB, pos[0])
                for h in (7, 6, 5, 4):
                    p2_norm(h, 1)
                p3_iblock(0)
                p2_head(0, 1, AB, pos[1])
                p3_iblock(1)
                p2_recip(1, 0, 4)
                p3_iblock(2)
                for h in (3, 2, 1, 0):
                    p2_norm(h, 1)
                p3_iblock(3)
                for i in range(4, NTT):
                    p3_iblock(i)
            s2.close()

    nc.compile()
    return nc


def get_nc():
    if "nc" not in _BUILD_CACHE:
        _BUILD_CACHE["nc"] = _build_nc()
    return _BUILD_CACHE["nc"]


def prepare_in_maps(x, wq, wk, wv, wo, q_norm_w, k_norm_w, cos, sin):
    import ml_dtypes
    bf = ml_dtypes.bfloat16

    x = np.asarray(x, np.float32)
    wq = np.asarray(wq, np.float32)
    wk = np.asarray(wk, np.float32)
    wv = np.asarray(wv, np.float32)
    wo = np.asarray(wo, np.float32)
    cos = np.asarray(cos, np.float32)
    sin = np.asarray(sin, np.float32)
    qw = np.asarray(q_norm_w, np.float32)
    kw = np.asarray(k_norm_w, np.float32)

    # rope tables in token-tile layout [128, NTT, ...]
    def tok_tiles(a):  # (T, F) -> (128, NTT, F)
        F = a.shape[1]
        return np.ascontiguousarray(
            a.reshape(NTT, 128, F).transpose(1, 0, 2)).astype(bf)

    tabaq = tok_tiles(cos * qw[None, :])
    tabak = tok_tiles(cos * kw[None, :])
    tabbqe = tok_tiles(-sin[:, 0::2] * qw[None, 1::2])
    tabbqo = tok_tiles(sin[:, 1::2] * qw[None, 0::2])
    tabbke = tok_tiles(-sin[:, 0::2] * kw[None, 1::2])
    tabbko = tok_tiles(sin[:, 1::2] * kw[None, 0::2])

    # x: [128, NTT, KT, 128] per batch (contraction tiles on partitions)
    xts = []
    for b in range(B):
        xT = x[b].T  # (D, T)
        # (KT,128,T) -> partitions first, then token tiles contiguous
        t1 = xT.reshape(KT, 128, NTT, 128)
        xts.append(np.ascontiguousarray(
            t1.transpose(1, 2, 0, 3)).astype(bf))

    in_maps = []
    wcache = {}
    for c in range(NCORES):
        b, g = divmod(c, G)
        if g not in wcache:
            # W columns: q heads g*8..g*8+7, then k0,k1, v0,v1 (96 each)
            cols = [wq[(g * QH + i) * HD:(g * QH + i + 1) * HD]
                    for i in range(QH)]
            cols += [wk[(g * KVH + i) * HD:(g * KVH + i + 1) * HD]
                     for i in range(KVH)]
            cols += [wv[(g * KVH + i) * HD:(g * KVH + i + 1) * HD]
                     for i in range(KVH)]
            wall = np.concatenate(cols, axis=0).T  # (D, 1152)
            w_t = np.ascontiguousarray(
                wall.reshape(KT, 128, PRJ).transpose(1, 0, 2)).astype(bf)
            # wot: rows = packed [768] head dims, cols = D
            wo_sh = wo[:, g * QH * HD:(g + 1) * QH * HD]  # (D, 768)
            wot = np.ascontiguousarray(
                wo_sh.T.reshape(6, 128, D).transpose(1, 0, 2)).astype(bf)
            wcache[g] = (w_t, wot)
        w_t, wot = wcache[g]
        in_maps.append({
            "xt": xts[b], "w": w_t, "wot": wot,
            "tabaq": tabaq, "tabbqe": tabbqe, "tabbqo": tabbqo,
            "tabak": tabak, "tabbke": tabbke, "tabbko": tabbko,
        })
    return in_maps


def kernel(**inputs):
    from concourse import bass_utils

    nc = get_nc()
    in_maps = prepare_in_maps(
        inputs["x"], inputs["wq"], inputs["wk"], inputs["wv"], inputs["wo"],
        inputs["q_norm_w"], inputs["k_norm_w"], inputs["cos"], inputs["sin"])
    trace = bool(int(os.environ.get("BASS_KERNEL_TRACE", "0")))
    res = bass_utils.run_bass_kernel_spmd(
        nc, in_maps, core_ids=list(range(NCORES)), trace=trace)
    _BUILD_CACHE["last_result"] = res
    partials = [np.asarray(r["out"], np.float32) for r in res.results]
    out = np.empty((B, T, D), np.float32)
    for b in range(B):
        out[b] = np.sum(np.stack(partials[b * G:(b + 1) * G]), axis=0,
                        dtype=np.float64).astype(np.float32)
    return out


# revision 42
# speedup vs baseline: 1.8290x; 1.0085x over previous
"""Grouped-Query Attention block (RMSNorm + RoPE + causal GQA + o_proj) on 8 trn2 NeuronCores.

Sharding: data-parallel over batch (2) x tensor-parallel over kv-head groups (4).
Core c = b*4 + g handles batch b, kv heads {2g, 2g+1}, q heads {8g..8g+7}.
Each core computes a partial o_proj output (T, D) over its 768 head-dims;
host sums the 4 group partials per batch.

v2 design (vs v1):
  * Phase 1 projections in token-on-partition layout: x tile is the
    stationary operand, all 12 head outputs (8q+2k+2v = 1152 dims) are the
    moving free dim in 3 dense 384-wide chunks -> 25% fewer PE cycles than
    per-head padded outputs, and RMSNorm/RoPE become cheap free-dim ops
    (per-token scalars live on partitions: tensor_scalar / [128,10]
    reciprocals instead of [1,512] single-lane reciprocals + PE broadcasts).
  * bf16 everywhere on the matmul path (same PE rate as fp32r at these
    free sizes, half the DMA/SBUF traffic).
  * q/k transposed back to [head_dim, T] via PE transpose-mode (96x128).
  * v needs no transpose at all in this layout (it was 16 PE transposes in v1).
  * Attention: scores transposed (k on partitions) with softmax k-sum folded
    into PV via per-head ones-columns appended to V at column 96+h, so each
    head's denominator lands on its own partition -> batched [8,512]
    reciprocal instead of 16 serial [1,512] ones (120us of DVE in v1).
  * Causal mask applied structurally (skip above-diagonal k-tiles, gpsimd
    affine_select on the rest) - same as v1.
  * Phase 3 o_proj over a densely packed [768 = 6x128, T] activation
    (head boundaries straddle tiles; packing via SBUF-SBUF DMA) -> 6
    contraction tiles instead of 8 per output tile: 25% fewer PE cycles.
  * Emission is software-pipelined so the PE queue never head-blocks on
    the ACT/DVE/gpsimd post-processing chains: transposes for token-tile
    tt are emitted after the projections of tt+1; attention q-chunk 0 is
    interleaved into the phase-1 tail; o_proj i-blocks are interleaved
    into attention q-chunk 1.
"""

import os
import sys

import numpy as np

sys.path.insert(0, "/opt/trn_rl_repo")

B, T, D = 2, 1024, 3072
NH, NKV, HD = 32, 8, 96
G = 4                 # tensor-parallel groups
QH = NH // G          # q heads per core (8)
KVH = NKV // G        # kv heads per core (2)
NCORES = 8
EPS = 1e-6
SCALE = 1.0 / float(np.sqrt(HD))
KT = D // 128          # 24 contraction tiles over d_model
NTT = T // 128         # 8 token tiles
QC = 2                 # q chunks in phase 2
QCS = T // QC          # 512
NJ = D // 512          # 6 o_proj output column chunks
RH = QH + KVH          # 10 rope heads (8 q + 2 k)
PRJ = QH * HD + KVH * HD * 2   # 1152 projected dims per core
NCH = 3                # projection chunks
CHW = PRJ // NCH       # 384
VCOL = HD + QH         # 104: v columns + per-head ones columns

_BUILD_CACHE = {}


def _build_nc():
    from contextlib import ExitStack
    from concourse import bacc, tile, mybir

    f32 = mybir.dt.float32
    f32r = mybir.dt.float32r
    bf16 = mybir.dt.bfloat16
    AF = mybir.ActivationFunctionType
    ALU = mybir.AluOpType

    nc = bacc.Bacc("TRN2", target_bir_lowering=False, debug=False,
                   num_devices=NCORES)

    xt_d = nc.dram_tensor("xt", (128, NTT, KT, 128), bf16, kind="ExternalInput").ap()
    w_d = nc.dram_tensor("w", (128, KT, PRJ), bf16, kind="ExternalInput").ap()
    wot_d = nc.dram_tensor("wot", (128, 6, D), bf16, kind="ExternalInput").ap()
    tabaq_d = nc.dram_tensor("tabaq", (128, NTT, HD), bf16, kind="ExternalInput").ap()
    tabbqe_d = nc.dram_tensor("tabbqe", (128, NTT, HD // 2), bf16, kind="ExternalInput").ap()
    tabbqo_d = nc.dram_tensor("tabbqo", (128, NTT, HD // 2), bf16, kind="ExternalInput").ap()
    tabak_d = nc.dram_tensor("tabak", (128, NTT, HD), bf16, kind="ExternalInput").ap()
    tabbke_d = nc.dram_tensor("tabbke", (128, NTT, HD // 2), bf16, kind="ExternalInput").ap()
    tabbko_d = nc.dram_tensor("tabbko", (128, NTT, HD // 2), bf16, kind="ExternalInput").ap()
    out_d = nc.dram_tensor("out", (T, D), bf16, kind="ExternalOutput").ap()

    with tile.TileContext(nc) as tc:
        with nc.allow_low_precision(reason="bf16 matmul path, fp32 accum"), \
             ExitStack() as ctx:
            const = ctx.enter_context(tc.tile_pool(name="const", bufs=1))

            ident = const.tile([128, 128], bf16, tag="ident")
            from concourse.masks import make_identity
            make_identity(nc, ident[:])
            eps_t = const.tile([128, 1], f32, tag="eps")
            nc.vector.memset(eps_t[:], EPS)
            ones_t = const.tile([128, HD], bf16, tag="ones_t")
            nc.vector.memset(ones_t[:], 1.0)

            # rope/norm tables (token-tile layout); DMA issues deferred
            tabs = ctx.enter_context(tc.tile_pool(name="tabs", bufs=1))
            tabaq = tabs.tile([128, NTT, HD], bf16, tag="tabaq")
            tabbqe = tabs.tile([128, NTT, HD // 2], bf16, tag="tabbqe")
            tabbqo = tabs.tile([128, NTT, HD // 2], bf16, tag="tabbqo")
            tabak = tabs.tile([128, NTT, HD], bf16, tag="tabak")
            tabbke = tabs.tile([128, NTT, HD // 2], bf16, tag="tabbke")
            tabbko = tabs.tile([128, NTT, HD // 2], bf16, tag="tabbko")

            def tab_dmas():
                for t, td in ((tabaq, tabaq_d), (tabbqe, tabbqe_d),
                              (tabbqo, tabbqo_d), (tabak, tabak_d),
                              (tabbke, tabbke_d), (tabbko, tabbko_d)):
                    nc.sync.dma_start(t[:], td[:])

            # weights resident in SBUF.  DMA issue order matters: the W
            # chunks are issued interleaved with the first token tiles so
            # the first projection matmuls are not stuck behind 12MB of
            # weight traffic; wot (only needed by o_proj, >200us in) is
            # issued mid-phase-1.
            wpool = ctx.enter_context(tc.tile_pool(name="wpool", bufs=1))
            w_sb = wpool.tile([128, KT, PRJ], bf16, tag="w_sb")
            wot_sb = wpool.tile([128, 6, D], bf16, tag="wot_sb")
            W_CHUNKS = [(0, 3), (3, 9), (9, 17), (17, KT)]

            def w_dma(ci):
                a, b = W_CHUNKS[ci]
                nc.sync.dma_start(w_sb[:, a:b, :], w_d[:, a:b, :])

            def wot_dma():
                nc.sync.dma_start(wot_sb[:], wot_d[:])

            # persistent activations
            p_act = ctx.enter_context(tc.tile_pool(name="p_act", bufs=1))
            qt = [p_act.tile([HD, T], bf16, tag=f"qt{h}", name=f"qt{h}")
                  for h in range(QH)]
            ktl = [p_act.tile([HD, T], bf16, tag=f"kt{g2}", name=f"kt{g2}")
                   for g2 in range(KVH)]
            vext = [p_act.tile([128, NTT, VCOL], bf16, tag=f"vx{g2}",
                               name=f"vx{g2}") for g2 in range(KVH)]
            for g2 in range(KVH):
                nc.vector.memset(vext[g2][:, :, HD:VCOL], 1.0)
            at_dense = [p_act.tile([128, 6, QCS], bf16, tag=f"at{qc}",
                                   name=f"at{qc}") for qc in range(QC)]
            sums_sb = p_act.tile([128, T], f32r, tag="sums_sb")
            rinv_sb = p_act.tile([128, T], bf16, tag="rinv_sb")
            # per-qc staging of rinv rows at partition 0 (reused across qc)
            rl8 = p_act.tile([1, QH, QCS], bf16, tag="rl8")
            amun = p_act.tile([128, QC * QH, QCS], bf16, tag="amun")

            # ---------------- pools ----------------
            # pool release must be LIFO: s2 (lives longest) is created
            # before s1 (phase-1 scope, closed first).
            s2 = ExitStack()
            sc_pool = s2.enter_context(
                tc.tile_pool(name="sc", bufs=2, space="PSUM"))
            po_pool = s2.enter_context(
                tc.tile_pool(name="po", bufs=1, space="PSUM"))
            pox_pool = s2.enter_context(
                tc.tile_pool(name="pox", bufs=1, space="PSUM"))
            pt_pool = s2.enter_context(tc.tile_pool(name="pt", bufs=3))
            am_pool = s2.enter_context(tc.tile_pool(name="am", bufs=2))

            s1 = ExitStack()
            xt_pool = s1.enter_context(tc.tile_pool(name="xt", bufs=2))
            ps1_pool = s1.enter_context(
                tc.tile_pool(name="ps1", bufs=1, space="PSUM"))
            pst_pool = s1.enter_context(
                tc.tile_pool(name="pst", bufs=1, space="PSUM"))
            w1_pool = s1.enter_context(tc.tile_pool(name="w1", bufs=2))

            # ---------------- phase 1 pieces ----------------
            psx_tiles = {}

            xt_tiles = {}

            def p1_xt(tt):
                xt_t = xt_pool.tile([128, KT, 128], bf16, tag="xt_t",
                                    name=f"xt{tt}")
                nc.sync.dma_start(xt_t[:], xt_d[:, tt])
                xt_tiles[tt] = xt_t

            def p1_mms(tt):
                xt_t = xt_tiles[tt]
                pss = []
                for c in range(NCH):
                    ps = ps1_pool.tile([128, CHW], f32, tag=f"ps{c}",
                                       name=f"ps{c}_{tt}")
                    pss.append(ps)
                for kt in range(KT):
                    for c in range(NCH):
                        nc.tensor.matmul(
                            pss[c][:],
                            xt_t[:, kt, :],
                            w_sb[:, kt, c * CHW:(c + 1) * CHW],
                            start=(kt == 0), stop=(kt == KT - 1))
                psx_tiles[tt] = pss

            roped_tiles = {}

            def p1_post(tt):
                pss = psx_tiles[tt]
                # copy psum -> sbuf bf16 (frees psum for next tt quickly)
                prj = w1_pool.tile([128, NCH, CHW], bf16, tag="prj",
                                   name=f"prj{tt}")
                for c in range(NCH):
                    nc.scalar.copy(prj[:, c, :], pss[c][:])
                # v heads: straight copy into vext (token-layout, no transpose)
                for g2 in range(KVH):
                    nc.gpsimd.tensor_copy(
                        vext[g2][:, tt, 0:HD],
                        prj[:, 2, HD * (KVH + g2):HD * (KVH + g2 + 1)])
                # rms: per-head sum of squares along free dim
                ssq = w1_pool.tile([128, RH], f32, tag="ssq", name=f"ssq{tt}")
                sqs = w1_pool.tile([128, HD], bf16, tag="sqs", name=f"sqs{tt}")
                for rh in range(RH):
                    c, off = divmod(rh * HD, CHW)
                    sl = prj[:, c, off:off + HD]
                    nc.vector.scalar_tensor_tensor(
                        sqs[:], sl, 1.0, sl,
                        op0=ALU.mult, op1=ALU.mult,
                        accum_out=ssq[:, rh:rh + 1])
                rms = w1_pool.tile([128, RH], f32, tag="rms", name=f"rms{tt}")
                nc.scalar.activation(rms[:], ssq[:], AF.Sqrt,
                                     bias=eps_t[:], scale=1.0 / HD)
                rinv = w1_pool.tile([128, RH], bf16, tag="rinv",
                                    name=f"rinv{tt}")
                nc.vector.reciprocal(rinv[:], rms[:])
                # rope (batched across heads per chunk)
                roped = w1_pool.tile([128, RH * HD], bf16, tag="roped",
                                     name=f"roped{tt}")
                tm0 = w1_pool.tile([128, CHW], bf16, tag="tm0", name=f"tm0{tt}")
                tm1 = w1_pool.tile([128, CHW], bf16, tag="tm1", name=f"tm1{tt}")
                tm2 = w1_pool.tile([128, CHW], bf16, tag="tm2", name=f"tm2{tt}")
                for c in range(NCH):
                    nh = 4 if c < 2 else 2
                    w96 = nh * HD
                    taba, tbe, tbo = ((tabaq, tabbqe, tabbqo) if c < 2
                                      else (tabak, tabbke, tabbko))
                    src = prj[:, c, 0:w96].rearrange("p (h d) -> p h d", h=nh)
                    rb = (rinv[:, c * 4:c * 4 + nh]
                          .unsqueeze(2).broadcast_to([128, nh, HD]))
                    t0 = tm0[:, 0:w96].rearrange("p (h d) -> p h d", h=nh)
                    nc.vector.tensor_mul(t0, src, rb)
                    ta = (taba[:, tt, :].unsqueeze(1)
                          .broadcast_to([128, nh, HD]))
                    t1 = tm1[:, 0:w96].rearrange("p (h d) -> p h d", h=nh)
                    nc.vector.tensor_mul(t1, t0, ta)
                    t0p = tm0[:, 0:w96].rearrange("p (h i e) -> p h i e",
                                                  h=nh, e=2)
                    t2p = tm2[:, 0:w96].rearrange("p (h i e) -> p h i e",
                                                  h=nh, e=2)
                    tbe_b = (tbe[:, tt, :].unsqueeze(1)
                             .broadcast_to([128, nh, HD // 2]))
                    tbo_b = (tbo[:, tt, :].unsqueeze(1)
                             .broadcast_to([128, nh, HD // 2]))
                    nc.vector.tensor_mul(t2p[:, :, :, 0], t0p[:, :, :, 1],
                                         tbe_b)
                    nc.vector.tensor_mul(t2p[:, :, :, 1], t0p[:, :, :, 0],
                                         tbo_b)
                    dst = roped[:, c * CHW:c * CHW + w96]
                    nc.vector.tensor_add(dst, tm1[:, 0:w96], tm2[:, 0:w96])
                roped_tiles[tt] = roped

            def p1_transposes(tt):
                roped = roped_tiles[tt]
                for rh in range(RH):
                    pst = pst_pool.tile([128, 128], bf16, tag="pst",
                                        name=f"pst{tt}_{rh}")
                    nc.tensor.transpose(
                        pst[0:HD, :],
                        roped[:, rh * HD:(rh + 1) * HD],
                        ident[:])
                    dst = qt[rh] if rh < QH else ktl[rh - QH]
                    nc.scalar.copy(dst[:, tt * 128:(tt + 1) * 128],
                                   pst[0:HD, :])

            # ---------------- phase 2 pieces ----------------
            # score tiles are trimmed to the causal region: for k-tile kt2
            # only q >= kt2*128 can attend, so the q-range of every score/
            # exp/PV op starts at max(qc*QCS, kt2*128).  The masked
            # diagonal triangle is always the first 128 columns of the
            # trimmed tile (base 0 affine_select).
            def p2_head(h, qc, pools=None, pop=None):
                g2 = h // (QH // KVH)
                nkt = (qc + 1) * QCS // 128
                po = (pop or po_pool).tile([128, QCS], f32, tag="po",
                                           name=f"po{qc}_{h}")
                pts = {}
                scp = pools or [sc_pool]

                def qoff_of(kt2):
                    return max(0, kt2 * 128 - qc * QCS)

                def sc_mm(kt2):
                    qoff = qoff_of(kt2)
                    tw = QCS - qoff
                    sc = scp[kt2 % len(scp)].tile(
                        [128, QCS], f32, tag="sc",
                        name=f"sc{qc}_{h}_{kt2}")
                    nc.tensor.matmul(
                        sc[:, 0:tw],
                        ktl[g2][:, kt2 * 128:(kt2 + 1) * 128],
                        qt[h][:, qc * QCS + qoff:(qc + 1) * QCS],
                        start=True, stop=True)
                    return sc

                def pt_make(kt2, sc):
                    qoff = qoff_of(kt2)
                    tw = QCS - qoff
                    pt = pt_pool.tile([128, QCS], bf16, tag="pt",
                                      name=f"pt{qc}_{h}_{kt2}")
                    nc.scalar.activation(pt[:, 0:tw], sc[:, 0:tw], AF.Exp,
                                         scale=SCALE)
                    if kt2 * 128 >= qc * QCS:
                        nc.gpsimd.affine_select(
                            pt[:, 0:128], pt[:, 0:128], pattern=[[1, 128]],
                            compare_op=ALU.is_ge,
                            fill=0.0,
                            base=0,
                            channel_multiplier=-1)
                    pts[kt2] = pt

                def pv_mm(kt2):
                    qoff = qoff_of(kt2)
                    tw = QCS - qoff
                    nc.tensor.matmul(
                        po[0:HD + h + 1, qoff:QCS],
                        vext[g2][:, kt2, 0:HD + h + 1],
                        pts[kt2][:, 0:tw],
                        start=(kt2 == 0), stop=(kt2 == nkt - 1),
                        skip_group_check=True)
                    pts[kt2] = None

                # software-pipelined: keep 2 sc tiles in flight
                sc0 = sc_mm(0)
                pt_make(0, sc0)
                for kt2 in range(1, nkt):
                    sc_n = sc_mm(kt2)
                    pv_mm(kt2 - 1)
                    pt_make(kt2, sc_n)
                pv_mm(nkt - 1)
                # denominator: po rows 96..96+h all hold this head's sum.
                # Engine partition starts must be 32-aligned, so copy the
                # whole [96:97+h] block; heads are processed in DESCENDING
                # order so each later (smaller) copy leaves row 96+h' of
                # earlier heads h' > h intact.
                qsl = slice(qc * QCS, (qc + 1) * QCS)
                nc.scalar.copy(sums_sb[HD:HD + h + 1, qsl],
                               po[HD:HD + h + 1, :])
                nc.vector.tensor_copy(amun[0:HD, qc * QH + h, :], po[0:HD, :])

            def p2_recip(qc, lo, hi):
                # engine partition starts must be 32-aligned: always start
                # at 96.  For the "hi" half (lo=4) the low rows are junk at
                # this point and get recomputed by the later lo pass.
                qsl = slice(qc * QCS, (qc + 1) * QCS)
                nc.vector.reciprocal(rinv_sb[HD:HD + hi, qsl],
                                     sums_sb[HD:HD + hi, qsl])
                # matmul lhsT/rhs base partition must be in {0,32,64}:
                # shift each head's rinv row down to partition 0 via DMA
                for h in range(lo, hi):
                    nc.sync.dma_start(rl8[0:1, h, :],
                                      rinv_sb[HD + h:HD + h + 1, qsl])

            def p2_norm(h, qc):
                bcs = am_pool.tile([HD, QCS], bf16, tag="bcs",
                                   name=f"bcs{qc}_{h}")
                nc.gpsimd.partition_broadcast(bcs[:], rl8[0:1, h, :],
                                              channels=HD)
                am = am_pool.tile([HD, QCS], bf16, tag="am",
                                  name=f"am{qc}_{h}")
                nc.vector.tensor_mul(am[:], amun[0:HD, qc * QH + h, :],
                                     bcs[:])
                # scatter into dense [768 = 6x128] layout (partition shift
                # is only legal via DMA)
                r0 = h * HD
                t0, off = divmod(r0, 128)
                n1 = min(128 - off, HD)
                nc.sync.dma_start(at_dense[qc][off:off + n1, t0, :],
                                  am[0:n1, :])
                if n1 < HD:
                    nc.sync.dma_start(
                        at_dense[qc][0:HD - n1, t0 + 1, :],
                        am[n1:HD, :])

            # ---------------- phase 3 pieces ----------------
            s3 = ExitStack()
            ps3_pool = None
            scx_pool = None
            ob_pool = None

            def p3_open():
                nonlocal ps3_pool, scx_pool, ob_pool
                ps3_pool = s3.enter_context(
                    tc.tile_pool(name="ps3", bufs=2, space="PSUM"))
                scx_pool = s3.enter_context(
                    tc.tile_pool(name="scx", bufs=2, space="PSUM"))
                ob_pool = s3.enter_context(tc.tile_pool(name="ob", bufs=1))

            def p3_iblock(i):
                qc = i // (QCS // 128)
                isl = slice((i % (QCS // 128)) * 128,
                            (i % (QCS // 128)) * 128 + 128)
                ob = ob_pool.tile([128, NJ, 512], bf16, tag="ob",
                                  name=f"ob{i}")
                for j in range(NJ):
                    ps3 = ps3_pool.tile([128, 512], f32, tag="ps3",
                                        name=f"ps3_{i}_{j}")
                    for t6 in range(6):
                        nc.tensor.matmul(
                            ps3[:], at_dense[qc][:, t6, isl],
                            wot_sb[:, t6, j * 512:(j + 1) * 512],
                            start=(t6 == 0), stop=(t6 == 5))
                    # alternate the psum->sbuf copies between DVE and ACT
                    # so neither queue gates the ps3 double-buffer rotation
                    if j % 2 == 0:
                        nc.vector.tensor_copy(ob[:, j, :], ps3[:])
                    else:
                        nc.scalar.copy(ob[:, j, :], ps3[:])
                    # store per j-chunk so the single ob buffer never gates
                    # the next i-block behind one big 768KB DMA
                    nc.sync.dma_start(
                        out_d[i * 128:(i + 1) * 128, j * 512:(j + 1) * 512],
                        ob[:, j, :])

            # ---------------- emission schedule ----------------
            with s1:
                # DMA issue order: xt(0) first (small, unblocks first mms),
                # then W chunks (deps require emission before the mms that
                # read them), tables, then xt(tt) prefetched per iteration.
                p1_xt(0)
                for ci in range(len(W_CHUNKS)):
                    w_dma(ci)
                tab_dmas()
                for tt in range(4):
                    p1_xt(tt + 1)
                    p1_mms(tt)
                    if tt == 2:
                        wot_dma()
                    if tt > 0:
                        p1_transposes(tt - 1)
                    p1_post(tt)
                pos = [po_pool, pox_pool]
                # all 8 qc0 heads run inside the phase-1 window (2 per
                # token tile, descending for the sums-copy clobber rule),
                # their chains hidden behind the projection matmuls
                for tt in range(4, NTT):
                    if tt + 1 < NTT:
                        p1_xt(tt + 1)
                    p1_mms(tt)
                    p1_transposes(tt - 1)
                    p1_post(tt)
                    h = QH - 1 - 2 * (tt - 4)
                    p2_head(h, 0, pop=pos[0])
                    p2_head(h - 1, 0, pop=pos[1])
                p1_transposes(NTT - 1)
            # phase-1 psum freed; open phase-3 pools (+ extra score bufs)
            p3_open()
            with s3:
                AB = [sc_pool, scx_pool]
                p2_recip(0, 0, QH)
                p2_head(7, 1, AB, pos[0])
                for h in (7, 6, 5, 4):
                    p2_norm(h, 0)
                p2_head(6, 1, AB, pos[1])
                for h in (3, 2, 1, 0):
                    p2_norm(h, 0)
                p2_head(5, 1, AB, pos[0])
                p3_iblock(0)
                p2_head(4, 1, AB, pos[1])
                p3_iblock(1)
                p2_recip(1, 4, QH)
                p2_head(3, 1, AB, pos[0])
                for h in (7, 6, 5, 4):
                    p2_norm(h, 1)
                p2_head(2, 1, AB, pos[1])
                p3_iblock(2)
                p2_head(1, 1, AB, pos[0])
                p3_iblock(3)
                p2_head(0, 1, AB, pos[1])
                p2_recip(1, 0, 4)
                for h in (3, 2, 1, 0):
                    p2_norm(h, 1)
                for i in range(4, NTT):
                    p3_iblock(i)
            s2.close()

    nc.compile()
    return nc


def get_nc():
    if "nc" not in _BUILD_CACHE:
        _BUILD_CACHE["nc"] = _build_nc()
    return _BUILD_CACHE["nc"]


def prepare_in_maps(x, wq, wk, wv, wo, q_norm_w, k_norm_w, cos, sin):
    import ml_dtypes
    bf = ml_dtypes.bfloat16

    x = np.asarray(x, np.float32)
    wq = np.asarray(wq, np.float32)
    wk = np.asarray(wk, np.float32)
    wv = np.asarray(wv, np.float32)
    wo = np.asarray(wo, np.float32)
    cos = np.asarray(cos, np.float32)
    sin = np.asarray(sin, np.float32)
    qw = np.asarray(q_norm_w, np.float32)
    kw = np.asarray(k_norm_w, np.float32)

    # rope tables in token-tile layout [128, NTT, ...]
    def tok_tiles(a):  # (T, F) -> (128, NTT, F)
        F = a.shape[1]
        return np.ascontiguousarray(
            a.reshape(NTT, 128, F).transpose(1, 0, 2)).astype(bf)

    tabaq = tok_tiles(cos * qw[None, :])
    tabak = tok_tiles(cos * kw[None, :])
    tabbqe = tok_tiles(-sin[:, 0::2] * qw[None, 1::2])
    tabbqo = tok_tiles(sin[:, 1::2] * qw[None, 0::2])
    tabbke = tok_tiles(-sin[:, 0::2] * kw[None, 1::2])
    tabbko = tok_tiles(sin[:, 1::2] * kw[None, 0::2])

    # x: [128, NTT, KT, 128] per batch (contraction tiles on partitions)
    xts = []
    for b in range(B):
        xT = x[b].T  # (D, T)
        # (KT,128,T) -> partitions first, then token tiles contiguous
        t1 = xT.reshape(KT, 128, NTT, 128)
        xts.append(np.ascontiguousarray(
            t1.transpose(1, 2, 0, 3)).astype(bf))

    in_maps = []
    wcache = {}
    for c in range(NCORES):
        b, g = divmod(c, G)
        if g not in wcache:
            # W columns: q heads g*8..g*8+7, then k0,k1, v0,v1 (96 each)
            cols = [wq[(g * QH + i) * HD:(g * QH + i + 1) * HD]
                    for i in range(QH)]
            cols += [wk[(g * KVH + i) * HD:(g * KVH + i + 1) * HD]
                     for i in range(KVH)]
            cols += [wv[(g * KVH + i) * HD:(g * KVH + i + 1) * HD]
                     for i in range(KVH)]
            wall = np.concatenate(cols, axis=0).T  # (D, 1152)
            w_t = np.ascontiguousarray(
                wall.reshape(KT, 128, PRJ).transpose(1, 0, 2)).astype(bf)
            # wot: rows = packed [768] head dims, cols = D
            wo_sh = wo[:, g * QH * HD:(g + 1) * QH * HD]  # (D, 768)
            wot = np.ascontiguousarray(
                wo_sh.T.reshape(6, 128, D).transpose(1, 0, 2)).astype(bf)
            wcache[g] = (w_t, wot)
        w_t, wot = wcache[g]
        in_maps.append({
            "xt": xts[b], "w": w_t, "wot": wot,
            "tabaq": tabaq, "tabbqe": tabbqe, "tabbqo": tabbqo,
            "tabak": tabak, "tabbke": tabbke, "tabbko": tabbko,
        })
    return in_maps


def kernel(**inputs):
    from concourse import bass_utils

    nc = get_nc()
    in_maps = prepare_in_maps(
        inputs["x"], inputs["wq"], inputs["wk"], inputs["wv"], inputs["wo"],
        inputs["q_norm_w"], inputs["k_norm_w"], inputs["cos"], inputs["sin"])
    trace = bool(int(os.environ.get("BASS_KERNEL_TRACE", "0")))
    res = bass_utils.run_bass_kernel_spmd(
        nc, in_maps, core_ids=list(range(NCORES)), trace=trace)
    _BUILD_CACHE["last_result"] = res
    partials = [np.asarray(r["out"], np.float32) for r in res.results]
    out = np.empty((B, T, D), np.float32)
    for b in range(B):
        out[b] = np.sum(np.stack(partials[b * G:(b + 1) * G]), axis=0,
                        dtype=np.float64).astype(np.float32)
    return out


# revision 44
# speedup vs baseline: 1.9019x; 1.0398x over previous
"""Grouped-Query Attention block (RMSNorm + RoPE + causal GQA + o_proj) on 8 trn2 NeuronCores.

Sharding: data-parallel over batch (2) x tensor-parallel over kv-head groups (4).
Core c = b*4 + g handles batch b, kv heads {2g, 2g+1}, q heads {8g..8g+7}.
Each core computes a partial o_proj output (T, D) over its 768 head-dims;
host sums the 4 group partials per batch.

v2 design (vs v1):
  * Phase 1 projections in token-on-partition layout: x tile is the
    stationary operand, all 12 head outputs (8q+2k+2v = 1152 dims) are the
    moving free dim in 3 dense 384-wide chunks -> 25% fewer PE cycles than
    per-head padded outputs, and RMSNorm/RoPE become cheap free-dim ops
    (per-token scalars live on partitions: tensor_scalar / [128,10]
    reciprocals instead of [1,512] single-lane reciprocals + PE broadcasts).
  * bf16 everywhere on the matmul path (same PE rate as fp32r at these
    free sizes, half the DMA/SBUF traffic).
  * q/k transposed back to [head_dim, T] via PE transpose-mode (96x128).
  * v needs no transpose at all in this layout (it was 16 PE transposes in v1).
  * Attention: scores transposed (k on partitions) with softmax k-sum folded
    into PV via per-head ones-columns appended to V at column 96+h, so each
    head's denominator lands on its own partition -> batched [8,512]
    reciprocal instead of 16 serial [1,512] ones (120us of DVE in v1).
  * Causal mask applied structurally (skip above-diagonal k-tiles, gpsimd
    affine_select on the rest) - same as v1.
  * Phase 3 o_proj over a densely packed [768 = 6x128, T] activation
    (head boundaries straddle tiles; packing via SBUF-SBUF DMA) -> 6
    contraction tiles instead of 8 per output tile: 25% fewer PE cycles.
  * Emission is software-pipelined so the PE queue never head-blocks on
    the ACT/DVE/gpsimd post-processing chains: transposes for token-tile
    tt are emitted after the projections of tt+1; attention q-chunk 0 is
    interleaved into the phase-1 tail; o_proj i-blocks are interleaved
    into attention q-chunk 1.
"""

import os
import sys

import numpy as np

sys.path.insert(0, "/opt/trn_rl_repo")

B, T, D = 2, 1024, 3072
NH, NKV, HD = 32, 8, 96
G = 4                 # tensor-parallel groups
QH = NH // G          # q heads per core (8)
KVH = NKV // G        # kv heads per core (2)
NCORES = 8
EPS = 1e-6
SCALE = 1.0 / float(np.sqrt(HD))
KT = D // 128          # 24 contraction tiles over d_model
NTT = T // 128         # 8 token tiles
QC = 2                 # q chunks in phase 2
QCS = T // QC          # 512
NJ = D // 512          # 6 o_proj output column chunks
RH = QH + KVH          # 10 rope heads (8 q + 2 k)
PRJ = QH * HD + KVH * HD * 2   # 1152 projected dims per core
NCH = 3                # projection chunks
CHW = PRJ // NCH       # 384
VCOL = HD + QH         # 104: v columns + per-head ones columns

_BUILD_CACHE = {}


def _build_nc():
    from contextlib import ExitStack
    from concourse import bacc, tile, mybir

    f32 = mybir.dt.float32
    f32r = mybir.dt.float32r
    bf16 = mybir.dt.bfloat16
    AF = mybir.ActivationFunctionType
    ALU = mybir.AluOpType

    nc = bacc.Bacc("TRN2", target_bir_lowering=False, debug=False,
                   num_devices=NCORES)

    xt_d = nc.dram_tensor("xt", (128, NTT, KT, 128), bf16, kind="ExternalInput").ap()
    w_d = nc.dram_tensor("w", (128, KT, PRJ), bf16, kind="ExternalInput").ap()
    wot_d = nc.dram_tensor("wot", (128, 6, D), bf16, kind="ExternalInput").ap()
    tabaq_d = nc.dram_tensor("tabaq", (128, NTT, HD), bf16, kind="ExternalInput").ap()
    tabbqe_d = nc.dram_tensor("tabbqe", (128, NTT, HD // 2), bf16, kind="ExternalInput").ap()
    tabbqo_d = nc.dram_tensor("tabbqo", (128, NTT, HD // 2), bf16, kind="ExternalInput").ap()
    tabak_d = nc.dram_tensor("tabak", (128, NTT, HD), bf16, kind="ExternalInput").ap()
    tabbke_d = nc.dram_tensor("tabbke", (128, NTT, HD // 2), bf16, kind="ExternalInput").ap()
    tabbko_d = nc.dram_tensor("tabbko", (128, NTT, HD // 2), bf16, kind="ExternalInput").ap()
    out_d = nc.dram_tensor("out", (T, D), bf16, kind="ExternalOutput").ap()

    with tile.TileContext(nc) as tc:
        with nc.allow_low_precision(reason="bf16 matmul path, fp32 accum"), \
             ExitStack() as ctx:
            const = ctx.enter_context(tc.tile_pool(name="const", bufs=1))

            ident = const.tile([128, 128], bf16, tag="ident")
            from concourse.masks import make_identity
            make_identity(nc, ident[:])
            eps_t = const.tile([128, 1], f32, tag="eps")
            nc.vector.memset(eps_t[:], EPS)
            ones_t = const.tile([128, HD], bf16, tag="ones_t")
            nc.vector.memset(ones_t[:], 1.0)

            # rope/norm tables (token-tile layout); DMA issues deferred
            tabs = ctx.enter_context(tc.tile_pool(name="tabs", bufs=1))
            tabaq = tabs.tile([128, NTT, HD], bf16, tag="tabaq")
            tabbqe = tabs.tile([128, NTT, HD // 2], bf16, tag="tabbqe")
            tabbqo = tabs.tile([128, NTT, HD // 2], bf16, tag="tabbqo")
            tabak = tabs.tile([128, NTT, HD], bf16, tag="tabak")
            tabbke = tabs.tile([128, NTT, HD // 2], bf16, tag="tabbke")
            tabbko = tabs.tile([128, NTT, HD // 2], bf16, tag="tabbko")

            def tab_dmas():
                for t, td in ((tabaq, tabaq_d), (tabbqe, tabbqe_d),
                              (tabbqo, tabbqo_d), (tabak, tabak_d),
                              (tabbke, tabbke_d), (tabbko, tabbko_d)):
                    nc.sync.dma_start(t[:], td[:])

            # weights resident in SBUF.  DMA issue order matters: the W
            # chunks are issued interleaved with the first token tiles so
            # the first projection matmuls are not stuck behind 12MB of
            # weight traffic; wot (only needed by o_proj, >200us in) is
            # issued mid-phase-1.
            wpool = ctx.enter_context(tc.tile_pool(name="wpool", bufs=1))
            w_sb = wpool.tile([128, KT, PRJ], bf16, tag="w_sb")
            wot_sb = wpool.tile([128, 6, D], bf16, tag="wot_sb")
            W_CHUNKS = [(0, 3), (3, 9), (9, 17), (17, KT)]

            def w_dma(ci):
                a, b = W_CHUNKS[ci]
                nc.sync.dma_start(w_sb[:, a:b, :], w_d[:, a:b, :])

            def wot_dma():
                nc.sync.dma_start(wot_sb[:], wot_d[:])

            # persistent activations
            p_act = ctx.enter_context(tc.tile_pool(name="p_act", bufs=1))
            qt = [p_act.tile([HD, T], bf16, tag=f"qt{h}", name=f"qt{h}")
                  for h in range(QH)]
            ktl = [p_act.tile([HD, T], bf16, tag=f"kt{g2}", name=f"kt{g2}")
                   for g2 in range(KVH)]
            vext = [p_act.tile([128, NTT, VCOL], bf16, tag=f"vx{g2}",
                               name=f"vx{g2}") for g2 in range(KVH)]
            for g2 in range(KVH):
                nc.vector.memset(vext[g2][:, :, HD:VCOL], 1.0)
            at_dense = [p_act.tile([128, 6, QCS], bf16, tag=f"at{qc}",
                                   name=f"at{qc}") for qc in range(QC)]
            sums_sb = p_act.tile([128, T], f32r, tag="sums_sb")
            rinv_sb = p_act.tile([128, T], bf16, tag="rinv_sb")
            # per-qc staging of rinv rows at partition 0 (reused across qc)
            rl8 = p_act.tile([1, QH, QCS], bf16, tag="rl8")
            amun = p_act.tile([128, QC * QH, QCS], bf16, tag="amun")

            # ---------------- pools ----------------
            # pool release must be LIFO: s2 (lives longest) is created
            # before s1 (phase-1 scope, closed first).
            s2 = ExitStack()
            sc_pool = s2.enter_context(
                tc.tile_pool(name="sc", bufs=2, space="PSUM"))
            po_pool = s2.enter_context(
                tc.tile_pool(name="po", bufs=1, space="PSUM"))
            pox_pool = s2.enter_context(
                tc.tile_pool(name="pox", bufs=1, space="PSUM"))
            pt_pool = s2.enter_context(tc.tile_pool(name="pt", bufs=4))
            am_pool = s2.enter_context(tc.tile_pool(name="am", bufs=2))

            s1 = ExitStack()
            xt_pool = s1.enter_context(tc.tile_pool(name="xt", bufs=2))
            ps1_pool = s1.enter_context(
                tc.tile_pool(name="ps1", bufs=1, space="PSUM"))
            pst_pool = s1.enter_context(
                tc.tile_pool(name="pst", bufs=1, space="PSUM"))
            w1_pool = s1.enter_context(tc.tile_pool(name="w1", bufs=2))

            # ---------------- phase 1 pieces ----------------
            psx_tiles = {}

            xt_tiles = {}

            def p1_xt(tt):
                xt_t = xt_pool.tile([128, KT, 128], bf16, tag="xt_t",
                                    name=f"xt{tt}")
                nc.sync.dma_start(xt_t[:], xt_d[:, tt])
                xt_tiles[tt] = xt_t

            def p1_mms(tt):
                xt_t = xt_tiles[tt]
                pss = []
                for c in range(NCH):
                    ps = ps1_pool.tile([128, CHW], f32, tag=f"ps{c}",
                                       name=f"ps{c}_{tt}")
                    pss.append(ps)
                for kt in range(KT):
                    for c in range(NCH):
                        nc.tensor.matmul(
                            pss[c][:],
                            xt_t[:, kt, :],
                            w_sb[:, kt, c * CHW:(c + 1) * CHW],
                            start=(kt == 0), stop=(kt == KT - 1))
                psx_tiles[tt] = pss

            roped_tiles = {}

            def p1_post(tt):
                pss = psx_tiles[tt]
                # copy psum -> sbuf bf16 (frees psum for next tt quickly)
                prj = w1_pool.tile([128, NCH, CHW], bf16, tag="prj",
                                   name=f"prj{tt}")
                for c in range(NCH):
                    nc.scalar.copy(prj[:, c, :], pss[c][:])
                # v heads: straight copy into vext (token-layout, no transpose)
                for g2 in range(KVH):
                    nc.gpsimd.tensor_copy(
                        vext[g2][:, tt, 0:HD],
                        prj[:, 2, HD * (KVH + g2):HD * (KVH + g2 + 1)])
                # rms: per-head sum of squares along free dim
                ssq = w1_pool.tile([128, RH], f32, tag="ssq", name=f"ssq{tt}")
                sqs = w1_pool.tile([128, HD], bf16, tag="sqs", name=f"sqs{tt}")
                for rh in range(RH):
                    c, off = divmod(rh * HD, CHW)
                    sl = prj[:, c, off:off + HD]
                    nc.vector.scalar_tensor_tensor(
                        sqs[:], sl, 1.0, sl,
                        op0=ALU.mult, op1=ALU.mult,
                        accum_out=ssq[:, rh:rh + 1])
                rms = w1_pool.tile([128, RH], f32, tag="rms", name=f"rms{tt}")
                nc.scalar.activation(rms[:], ssq[:], AF.Sqrt,
                                     bias=eps_t[:], scale=1.0 / HD)
                rinv = w1_pool.tile([128, RH], bf16, tag="rinv",
                                    name=f"rinv{tt}")
                nc.vector.reciprocal(rinv[:], rms[:])
                # rope (batched across heads per chunk)
                roped = w1_pool.tile([128, RH * HD], bf16, tag="roped",
                                     name=f"roped{tt}")
                tm0 = w1_pool.tile([128, CHW], bf16, tag="tm0", name=f"tm0{tt}")
                tm1 = w1_pool.tile([128, CHW], bf16, tag="tm1", name=f"tm1{tt}")
                tm2 = w1_pool.tile([128, CHW], bf16, tag="tm2", name=f"tm2{tt}")
                for c in range(NCH):
                    nh = 4 if c < 2 else 2
                    w96 = nh * HD
                    taba, tbe, tbo = ((tabaq, tabbqe, tabbqo) if c < 2
                                      else (tabak, tabbke, tabbko))
                    src = prj[:, c, 0:w96].rearrange("p (h d) -> p h d", h=nh)
                    rb = (rinv[:, c * 4:c * 4 + nh]
                          .unsqueeze(2).broadcast_to([128, nh, HD]))
                    t0 = tm0[:, 0:w96].rearrange("p (h d) -> p h d", h=nh)
                    nc.vector.tensor_mul(t0, src, rb)
                    ta = (taba[:, tt, :].unsqueeze(1)
                          .broadcast_to([128, nh, HD]))
                    t1 = tm1[:, 0:w96].rearrange("p (h d) -> p h d", h=nh)
                    nc.vector.tensor_mul(t1, t0, ta)
                    t0p = tm0[:, 0:w96].rearrange("p (h i e) -> p h i e",
                                                  h=nh, e=2)
                    t2p = tm2[:, 0:w96].rearrange("p (h i e) -> p h i e",
                                                  h=nh, e=2)
                    tbe_b = (tbe[:, tt, :].unsqueeze(1)
                             .broadcast_to([128, nh, HD // 2]))
                    tbo_b = (tbo[:, tt, :].unsqueeze(1)
                             .broadcast_to([128, nh, HD // 2]))
                    nc.vector.tensor_mul(t2p[:, :, :, 0], t0p[:, :, :, 1],
                                         tbe_b)
                    nc.vector.tensor_mul(t2p[:, :, :, 1], t0p[:, :, :, 0],
                                         tbo_b)
                    dst = roped[:, c * CHW:c * CHW + w96]
                    nc.vector.tensor_add(dst, tm1[:, 0:w96], tm2[:, 0:w96])
                roped_tiles[tt] = roped

            def p1_transposes(tt):
                roped = roped_tiles[tt]
                for rh in range(RH):
                    pst = pst_pool.tile([128, 128], bf16, tag="pst",
                                        name=f"pst{tt}_{rh}")
                    nc.tensor.transpose(
                        pst[0:HD, :],
                        roped[:, rh * HD:(rh + 1) * HD],
                        ident[:])
                    dst = qt[rh] if rh < QH else ktl[rh - QH]
                    nc.scalar.copy(dst[:, tt * 128:(tt + 1) * 128],
                                   pst[0:HD, :])

            # ---------------- phase 2 pieces ----------------
            # score tiles are trimmed to the causal region: for k-tile kt2
            # only q >= kt2*128 can attend, so the q-range of every score/
            # exp/PV op starts at max(qc*QCS, kt2*128).  The masked
            # diagonal triangle is always the first 128 columns of the
            # trimmed tile (base 0 affine_select).
            def p2_head(h, qc, pools=None, pop=None):
                g2 = h // (QH // KVH)
                nkt = (qc + 1) * QCS // 128
                po = (pop or po_pool).tile([128, QCS], f32, tag="po",
                                           name=f"po{qc}_{h}")
                pts = {}
                scp = pools or [sc_pool]

                def qoff_of(kt2):
                    return max(0, kt2 * 128 - qc * QCS)

                def sc_mm(kt2):
                    qoff = qoff_of(kt2)
                    tw = QCS - qoff
                    sc = scp[kt2 % len(scp)].tile(
                        [128, QCS], f32, tag="sc",
                        name=f"sc{qc}_{h}_{kt2}")
                    nc.tensor.matmul(
                        sc[:, 0:tw],
                        ktl[g2][:, kt2 * 128:(kt2 + 1) * 128],
                        qt[h][:, qc * QCS + qoff:(qc + 1) * QCS],
                        start=True, stop=True)
                    return sc

                def pt_make(kt2, sc):
                    qoff = qoff_of(kt2)
                    tw = QCS - qoff
                    pt = pt_pool.tile([128, QCS], bf16, tag="pt",
                                      name=f"pt{qc}_{h}_{kt2}")
                    nc.scalar.activation(pt[:, 0:tw], sc[:, 0:tw], AF.Exp,
                                         scale=SCALE)
                    if kt2 * 128 >= qc * QCS:
                        nc.gpsimd.affine_select(
                            pt[:, 0:128], pt[:, 0:128], pattern=[[1, 128]],
                            compare_op=ALU.is_ge,
                            fill=0.0,
                            base=0,
                            channel_multiplier=-1)
                    pts[kt2] = pt

                def pv_mm(kt2):
                    qoff = qoff_of(kt2)
                    tw = QCS - qoff
                    nc.tensor.matmul(
                        po[0:HD + h + 1, qoff:QCS],
                        vext[g2][:, kt2, 0:HD + h + 1],
                        pts[kt2][:, 0:tw],
                        start=(kt2 == 0), stop=(kt2 == nkt - 1),
                        skip_group_check=True)
                    pts[kt2] = None

                # software-pipelined: keep 2 sc tiles in flight
                sc0 = sc_mm(0)
                pt_make(0, sc0)
                for kt2 in range(1, nkt):
                    sc_n = sc_mm(kt2)
                    pv_mm(kt2 - 1)
                    pt_make(kt2, sc_n)
                pv_mm(nkt - 1)
                # denominator: po rows 96..96+h all hold this head's sum.
                # Engine partition starts must be 32-aligned, so copy the
                # whole [96:97+h] block; heads are processed in DESCENDING
                # order so each later (smaller) copy leaves row 96+h' of
                # earlier heads h' > h intact.
                qsl = slice(qc * QCS, (qc + 1) * QCS)
                nc.scalar.copy(sums_sb[HD:HD + h + 1, qsl],
                               po[HD:HD + h + 1, :])
                nc.vector.tensor_copy(amun[0:HD, qc * QH + h, :], po[0:HD, :])

            def p2_recip(qc, lo, hi):
                # engine partition starts must be 32-aligned: always start
                # at 96.  For the "hi" half (lo=4) the low rows are junk at
                # this point and get recomputed by the later lo pass.
                qsl = slice(qc * QCS, (qc + 1) * QCS)
                nc.vector.reciprocal(rinv_sb[HD:HD + hi, qsl],
                                     sums_sb[HD:HD + hi, qsl])
                # matmul lhsT/rhs base partition must be in {0,32,64}:
                # shift each head's rinv row down to partition 0 via DMA
                for h in range(lo, hi):
                    nc.sync.dma_start(rl8[0:1, h, :],
                                      rinv_sb[HD + h:HD + h + 1, qsl])

            def p2_norm(h, qc):
                bcs = am_pool.tile([HD, QCS], bf16, tag="bcs",
                                   name=f"bcs{qc}_{h}")
                nc.gpsimd.partition_broadcast(bcs[:], rl8[0:1, h, :],
                                              channels=HD)
                am = am_pool.tile([HD, QCS], bf16, tag="am",
                                  name=f"am{qc}_{h}")
                nc.vector.tensor_mul(am[:], amun[0:HD, qc * QH + h, :],
                                     bcs[:])
                # scatter into dense [768 = 6x128] layout (partition shift
                # is only legal via DMA)
                r0 = h * HD
                t0, off = divmod(r0, 128)
                n1 = min(128 - off, HD)
                nc.sync.dma_start(at_dense[qc][off:off + n1, t0, :],
                                  am[0:n1, :])
                if n1 < HD:
                    nc.sync.dma_start(
                        at_dense[qc][0:HD - n1, t0 + 1, :],
                        am[n1:HD, :])

            # ---------------- phase 3 pieces ----------------
            s3 = ExitStack()
            ps3_pool = None
            scx_pool = None
            ob_pool = None

            def p3_open():
                nonlocal ps3_pool, scx_pool, ob_pool
                ps3_pool = s3.enter_context(
                    tc.tile_pool(name="ps3", bufs=2, space="PSUM"))
                scx_pool = s3.enter_context(
                    tc.tile_pool(name="scx", bufs=2, space="PSUM"))
                ob_pool = s3.enter_context(tc.tile_pool(name="ob", bufs=1))

            def p3_iblock(i):
                qc = i // (QCS // 128)
                isl = slice((i % (QCS // 128)) * 128,
                            (i % (QCS // 128)) * 128 + 128)
                ob = ob_pool.tile([128, NJ, 512], bf16, tag="ob",
                                  name=f"ob{i}")
                for j in range(NJ):
                    ps3 = ps3_pool.tile([128, 512], f32, tag="ps3",
                                        name=f"ps3_{i}_{j}")
                    for t6 in range(6):
                        nc.tensor.matmul(
                            ps3[:], at_dense[qc][:, t6, isl],
                            wot_sb[:, t6, j * 512:(j + 1) * 512],
                            start=(t6 == 0), stop=(t6 == 5))
                    # alternate the psum->sbuf copies between DVE and ACT
                    # so neither queue gates the ps3 double-buffer rotation
                    if j % 2 == 0:
                        nc.vector.tensor_copy(ob[:, j, :], ps3[:])
                    else:
                        nc.scalar.copy(ob[:, j, :], ps3[:])
                    # store per j-chunk so the single ob buffer never gates
                    # the next i-block behind one big 768KB DMA
                    nc.sync.dma_start(
                        out_d[i * 128:(i + 1) * 128, j * 512:(j + 1) * 512],
                        ob[:, j, :])

            # ---------------- emission schedule ----------------
            with s1:
                # DMA issue order: xt(0) first (small, unblocks first mms),
                # then W chunks (deps require emission before the mms that
                # read them), tables, then xt(tt) prefetched per iteration.
                p1_xt(0)
                for ci in range(len(W_CHUNKS)):
                    w_dma(ci)
                tab_dmas()
                for tt in range(4):
                    p1_xt(tt + 1)
                    p1_mms(tt)
                    if tt == 2:
                        wot_dma()
                    if tt > 0:
                        p1_transposes(tt - 1)
                    p1_post(tt)
                pos = [po_pool, pox_pool]
                # all 8 qc0 heads run inside the phase-1 window (2 per
                # token tile, descending for the sums-copy clobber rule),
                # their chains hidden behind the projection matmuls
                for tt in range(4, NTT):
                    if tt + 1 < NTT:
                        p1_xt(tt + 1)
                    p1_mms(tt)
                    p1_transposes(tt - 1)
                    p1_post(tt)
                    h = QH - 1 - 2 * (tt - 4)
                    p2_head(h, 0, pop=pos[0])
                    p2_head(h - 1, 0, pop=pos[1])
                p1_transposes(NTT - 1)
            # phase-1 psum freed; open phase-3 pools (+ extra score bufs)
            p3_open()
            with s3:
                AB = [sc_pool, scx_pool]
                p2_recip(0, 0, QH)
                p2_head(7, 1, AB, pos[0])
                for h in (7, 6, 5, 4):
                    p2_norm(h, 0)
                p2_head(6, 1, AB, pos[1])
                for h in (3, 2, 1, 0):
                    p2_norm(h, 0)
                p2_head(5, 1, AB, pos[0])
                p3_iblock(0)
                p2_head(4, 1, AB, pos[1])
                p3_iblock(1)
                p2_recip(1, 4, QH)
                p2_head(3, 1, AB, pos[0])
                for h in (7, 6, 5, 4):
                    p2_norm(h, 1)
                p2_head(2, 1, AB, pos[1])
                p2_head(1, 1, AB, pos[0])
                p2_head(0, 1, AB, pos[1])
                p2_recip(1, 0, 4)
                # i2/i3 only need qc0 data: they fill the PE bubble while
                # the qc1 low-half normalize chain drains
                p3_iblock(2)
                for h in (3, 2, 1, 0):
                    p2_norm(h, 1)
                p3_iblock(3)
                for i in range(4, NTT):
                    p3_iblock(i)
            s2.close()

    nc.compile()
    return nc


def get_nc():
    if "nc" not in _BUILD_CACHE:
        _BUILD_CACHE["nc"] = _build_nc()
    return _BUILD_CACHE["nc"]


def prepare_in_maps(x, wq, wk, wv, wo, q_norm_w, k_norm_w, cos, sin):
    import ml_dtypes
    bf = ml_dtypes.bfloat16

    x = np.asarray(x, np.float32)
    wq = np.asarray(wq, np.float32)
    wk = np.asarray(wk, np.float32)
    wv = np.asarray(wv, np.float32)
    wo = np.asarray(wo, np.float32)
    cos = np.asarray(cos, np.float32)
    sin = np.asarray(sin, np.float32)
    qw = np.asarray(q_norm_w, np.float32)
    kw = np.asarray(k_norm_w, np.float32)

    # rope tables in token-tile layout [128, NTT, ...]
    def tok_tiles(a):  # (T, F) -> (128, NTT, F)
        F = a.shape[1]
        return np.ascontiguousarray(
            a.reshape(NTT, 128, F).transpose(1, 0, 2)).astype(bf)

    tabaq = tok_tiles(cos * qw[None, :])
    tabak = tok_tiles(cos * kw[None, :])
    tabbqe = tok_tiles(-sin[:, 0::2] * qw[None, 1::2])
    tabbqo = tok_tiles(sin[:, 1::2] * qw[None, 0::2])
    tabbke = tok_tiles(-sin[:, 0::2] * kw[None, 1::2])
    tabbko = tok_tiles(sin[:, 1::2] * kw[None, 0::2])

    # x: [128, NTT, KT, 128] per batch (contraction tiles on partitions)
    xts = []
    for b in range(B):
        xT = x[b].T  # (D, T)
        # (KT,128,T) -> partitions first, then token tiles contiguous
        t1 = xT.reshape(KT, 128, NTT, 128)
        xts.append(np.ascontiguousarray(
            t1.transpose(1, 2, 0, 3)).astype(bf))

    in_maps = []
    wcache = {}
    for c in range(NCORES):
        b, g = divmod(c, G)
        if g not in wcache:
            # W columns: q heads g*8..g*8+7, then k0,k1, v0,v1 (96 each)
            cols = [wq[(g * QH + i) * HD:(g * QH + i + 1) * HD]
                    for i in range(QH)]
            cols += [wk[(g * KVH + i) * HD:(g * KVH + i + 1) * HD]
                     for i in range(KVH)]
            cols += [wv[(g * KVH + i) * HD:(g * KVH + i + 1) * HD]
                     for i in range(KVH)]
            wall = np.concatenate(cols, axis=0).T  # (D, 1152)
            w_t = np.ascontiguousarray(
                wall.reshape(KT, 128, PRJ).transpose(1, 0, 2)).astype(bf)
            # wot: rows = packed [768] head dims, cols = D
            wo_sh = wo[:, g * QH * HD:(g + 1) * QH * HD]  # (D, 768)
            wot = np.ascontiguousarray(
                wo_sh.T.reshape(6, 128, D).transpose(1, 0, 2)).astype(bf)
            wcache[g] = (w_t, wot)
        w_t, wot = wcache[g]
        in_maps.append({
            "xt": xts[b], "w": w_t, "wot": wot,
            "tabaq": tabaq, "tabbqe": tabbqe, "tabbqo": tabbqo,
            "tabak": tabak, "tabbke": tabbke, "tabbko": tabbko,
        })
    return in_maps


def kernel(**inputs):
    from concourse import bass_utils

    nc = get_nc()
    in_maps = prepare_in_maps(
        inputs["x"], inputs["wq"], inputs["wk"], inputs["wv"], inputs["wo"],
        inputs["q_norm_w"], inputs["k_norm_w"], inputs["cos"], inputs["sin"])
    trace = bool(int(os.environ.get("BASS_KERNEL_TRACE", "0")))
    res = bass_utils.run_bass_kernel_spmd(
        nc, in_maps, core_ids=list(range(NCORES)), trace=trace)
    _BUILD_CACHE["last_result"] = res
    partials = [np.asarray(r["out"], np.float32) for r in res.results]
    out = np.empty((B, T, D), np.float32)
    for b in range(B):
        out[b] = np.sum(np.stack(partials[b * G:(b + 1) * G]), axis=0,
                        dtype=np.float64).astype(np.float32)
    return out
